# revision 1
# baseline (speedup 1.0000x reference)
"""Trainium2 Bass kernel for nn_Net_41807211660013 (PNA-style GNN + UMAP head).

Contract: kernel(**inputs) takes FULL unsharded inputs (as from
reference.setup_inputs()) and returns the FULL [8192, 8192] float32 output.

Strategy (8 NeuronCores, SPMD):
  - nodes sharded by id range: core c owns dst nodes [1024c, 1024(c+1))
  - host reorders each core's nodes by degree into 8 buckets of 128 lanes with
    per-bucket uniform padded degree D_b (pad slots repeat a real neighbor,
    sums corrected on device); all gather indices are in this permuted order
  - per layer: dma_gather of h rows from an HBM table -> [128, D_b, 128] f32
    tiles; segmented min/max/sum/sumsq via DVE tensor_reduce; folded W_post
    matmul on PE; BatchNorm stats via ones-matmul + AllReduce; residual;
    AllGather of the new h shard rebuilds the table
  - final: 3-layer MLP, AllGather of augmented y^T, row-sharded NxN distance
    matmul with p = 0.5 - 0.5*tanh(0.5*(B*ln(d2) + ln(A))) epilogue
  - host un-permutes output rows/cols
"""
import sys

if "/opt/trn_rl_repo" not in sys.path:
    sys.path.insert(0, "/opt/trn_rl_repo")

import numpy as np

N, E, F_IN, H, ED = 8192, 524288, 39, 80, 4
A_UMAP, B_UMAP = 0.583, 1.334
NC_ = 8
NPC = N // NC_            # 1024 nodes per core
NB = NPC // 128           # 8 buckets
ZERO_ROW = N              # table row of zeros
TABLE_ROWS = N + 1
NL = 4                    # message-passing layers
SQRT_EPS = float(np.sqrt(np.float32(1e-5)))
LN_A = float(np.log(np.float32(A_UMAP)))

# const layout indices (consts tile is [128, 5*NB], slice [:, k*NB+b])
K_INVDEG, K_NPAD, K_AMP, K_ATT, K_HAS = range(5)

_cache = {}


# --------------------------------------------------------------------------
# host preprocessing
# --------------------------------------------------------------------------
def _prepare(inputs):
    x = np.asarray(inputs["x"], np.float32)
    edge_attr = np.asarray(inputs["edge_attr"], np.float32)
    edge_index = np.asarray(inputs["edge_index"], np.int64)
    src_arr, dst_arr = edge_index[0], edge_index[1]

    deg = np.bincount(dst_arr, minlength=N).astype(np.float32)
    logd = np.log1p(deg)
    avg_log = logd.mean(dtype=np.float32)
    amp = (logd / avg_log).astype(np.float32)
    att = np.where(logd > 0, avg_log / np.where(logd > 0, logd, 1.0), 1.0).astype(np.float32)
    has = (deg > 0).astype(np.float32)
    inv_deg = (1.0 / np.where(deg > 0, deg, 1.0)).astype(np.float32)

    order = np.argsort(dst_arr, kind="stable")
    sorted_src = src_arr[order]
    sorted_eid = order
    starts = np.searchsorted(dst_arr[order], np.arange(N))
    ends = np.searchsorted(dst_arr[order], np.arange(N) + 1)

    perm_nodes = np.empty(N, dtype=np.int64)
    for c in range(NC_):
        own = np.arange(c * NPC, (c + 1) * NPC)
        loc = own[np.argsort(-deg[own], kind="stable")]
        perm_nodes[c * NPC:(c + 1) * NPC] = loc
    perm_pos = np.empty(N, dtype=np.int64)
    perm_pos[perm_nodes] = np.arange(N)

    deg_perm = deg[perm_nodes].astype(np.int64).reshape(NC_, NB, 128)
    D = np.maximum(deg_perm.max(axis=(0, 2)), 1).astype(np.int64)  # [NB]
    G = int(D.sum())

    # slot structures per core
    idx_slots = np.full((NC_, G, 128), ZERO_ROW, dtype=np.int32)   # [c][slot][lane]
    eattr_p = np.zeros((NC_, 128, G, ED), dtype=np.float32)        # partition-major
    npad = np.zeros(N, dtype=np.float32)
    offs = np.concatenate([[0], np.cumsum(D)]).astype(np.int64)
    for c in range(NC_):
        for b in range(NB):
            Db, ob = int(D[b]), int(offs[b])
            for p in range(128):
                r = c * NPC + b * 128 + p
                n = perm_nodes[r]
                d = int(deg[n])
                if d == 0:
                    continue
                srcs = perm_pos[sorted_src[starts[n]:ends[n]]]
                eids = sorted_eid[starts[n]:ends[n]]
                idx_slots[c, ob:ob + d, p] = srcs
                idx_slots[c, ob + d:ob + Db, p] = srcs[0]
                ea = edge_attr[eids]
                eattr_p[c, p, ob:ob + d] = ea
                eattr_p[c, p, ob + d:ob + Db] = ea[0]
                npad[r] = Db - d

    # idx in dma_gather wrap layout: value for slot i lives at [i % 16, i // 16],
    # replicated over all 128 partitions
    idx_wrap = np.zeros((NC_, 128, G * 8), dtype=np.int16)
    for c in range(NC_):
        flat = idx_slots[c].reshape(G * 128)             # i = g*128 + lane
        w = flat.reshape(G * 8, 16).T.astype(np.int16)   # [16, G*8]
        idx_wrap[c] = np.tile(w, (8, 1))

    # per-(core, lane, bucket) consts [128, 5*NB]
    consts = np.zeros((NC_, 128, 5 * NB), dtype=np.float32)
    for c in range(NC_):
        rows = perm_nodes[c * NPC:(c + 1) * NPC].reshape(NB, 128)
        trows = np.arange(c * NPC, (c + 1) * NPC).reshape(NB, 128)
        for b in range(NB):
            consts[c, :, K_INVDEG * NB + b] = inv_deg[rows[b]]
            consts[c, :, K_NPAD * NB + b] = npad[trows[b]]
            consts[c, :, K_AMP * NB + b] = amp[rows[b]]
            consts[c, :, K_ATT * NB + b] = att[rows[b]]
            consts[c, :, K_HAS * NB + b] = has[rows[b]]

    # folded W_post weights
    W_post = np.asarray(inputs["W_post"], np.float32)
    b_post = np.asarray(inputs["b_post"], np.float32)
    W_eff = np.zeros((NL, 416, 240), dtype=np.float32)
    b_eff = np.zeros((NL, 3, 80), dtype=np.float32)
    for l in range(NL):
        W = W_post[l]
        for v in range(3):
            o = 656 * v
            W_hown = W[o + 0:o + 80] + W[o + 164:o + 244] + W[o + 328:o + 408]
            W_e = np.concatenate([W[o + 160:o + 164], W[o + 324:o + 328],
                                  W[o + 488:o + 492], W[o + 652:o + 656]], axis=0)
            W_eff[l, :, 80 * v:80 * (v + 1)] = np.concatenate(
                [W_hown, W[o + 80:o + 160], W[o + 244:o + 324],
                 W[o + 408:o + 488], W[o + 572:o + 652], W_e], axis=0)
            b_eff[l, v] = np.float32(SQRT_EPS) * W[o + 492:o + 572].sum(axis=0)
        b_eff[l, 0] += b_post[l]
    # pack for device: [128, NL, 4, 240] (K-chunk partition-major, zero padded)
    w_eff_packed = np.zeros((128, NL, 4, 240), dtype=np.float32)
    for l in range(NL):
        for kt in range(4):
            kw = min(128, 416 - kt * 128)
            w_eff_packed[:kw, l, kt] = W_eff[l, kt * 128:kt * 128 + kw]
    b_eff_r = np.broadcast_to(b_eff.reshape(1, NL * 3 * 80), (128, NL * 3 * 80)).copy()

    bn_gamma = np.asarray(inputs["bn_gamma"], np.float32)
    bn_beta = np.asarray(inputs["bn_beta"], np.float32)
    bn_r = np.broadcast_to(
        np.concatenate([bn_gamma.reshape(NL * 80), bn_beta.reshape(NL * 80)])
        .reshape(1, 2 * NL * 80), (128, 2 * NL * 80)).copy()

    xt = x.T[:, perm_nodes].copy()                 # [39, 8192] table order

    mlp_b_r = np.broadcast_to(
        np.concatenate([np.asarray(inputs["b1"], np.float32),
                        np.asarray(inputs["b2"], np.float32),
                        np.asarray(inputs["b3"], np.float32)]).reshape(1, 92),
        (128, 92)).copy()

    shared = dict(
        xt=np.ascontiguousarray(xt),
        w_emb=np.asarray(inputs["W_emb"], np.float32),
        b_emb_r=np.broadcast_to(np.asarray(inputs["b_emb"], np.float32)
                                .reshape(1, 80), (128, 80)).copy(),
        w_eff=w_eff_packed.reshape(128, NL * 4 * 240),
        b_eff_r=b_eff_r,
        bn_r=bn_r,
        w1=np.asarray(inputs["W1"], np.float32),
        w2=np.asarray(inputs["W2"], np.float32),
        w3=np.asarray(inputs["W3"], np.float32),
        mlp_b_r=mlp_b_r,
    )
    per_core = []
    for c in range(NC_):
        per_core.append(dict(
            xt_own=np.ascontiguousarray(xt[:, c * NPC:(c + 1) * NPC]),
            idx=idx_wrap[c],
            eattr=eattr_p[c].reshape(128, G * ED),
            consts=consts[c],
        ))
    return shared, per_core, perm_nodes, [int(d) for d in D]


# --------------------------------------------------------------------------
# device program
# --------------------------------------------------------------------------
def _build(D):
    import os
    NO_GATHER = os.environ.get("NO_GATHER") == "1"
    NO_CC = os.environ.get("NO_CC") == "1"
    NO_PB = os.environ.get("NO_PB") == "1"
    import concourse.bass as bass  # noqa: F401
    import concourse.bacc as bacc
    import concourse.tile as tile
    import concourse.mybir as mybir
    import concourse.masks as masks

    f32 = mybir.dt.float32
    i16 = mybir.dt.int16
    AF = mybir.ActivationFunctionType
    OP = mybir.AluOpType
    AX = mybir.AxisListType

    G = sum(D)
    offs = [0]
    for d in D:
        offs.append(offs[-1] + d)
    DMAX = max(D)

    nc = bacc.Bacc("TRN2", target_bir_lowering=False, debug=False,
                   num_devices=NC_)

    # ---- I/O ----
    xt_d = nc.dram_tensor("xt", [F_IN, N], f32, kind="ExternalInput")
    xt_own_d = nc.dram_tensor("xt_own", [F_IN, NPC], f32, kind="ExternalInput")
    w_emb_d = nc.dram_tensor("w_emb", [F_IN, H], f32, kind="ExternalInput")
    b_emb_d = nc.dram_tensor("b_emb_r", [128, H], f32, kind="ExternalInput")
    idx_d = nc.dram_tensor("idx", [128, G * 8], i16, kind="ExternalInput")
    eattr_d = nc.dram_tensor("eattr", [128, G * ED], f32, kind="ExternalInput")
    consts_d = nc.dram_tensor("consts", [128, 5 * NB], f32, kind="ExternalInput")
    w_eff_d = nc.dram_tensor("w_eff", [128, NL * 4 * 240], f32, kind="ExternalInput")
    b_eff_d = nc.dram_tensor("b_eff_r", [128, NL * 3 * 80], f32, kind="ExternalInput")
    bn_d = nc.dram_tensor("bn_r", [128, 2 * NL * 80], f32, kind="ExternalInput")
    w1_d = nc.dram_tensor("w1", [80, 40], f32, kind="ExternalInput")
    w2_d = nc.dram_tensor("w2", [40, 20], f32, kind="ExternalInput")
    w3_d = nc.dram_tensor("w3", [20, 32], f32, kind="ExternalInput")
    mlpb_d = nc.dram_tensor("mlp_b_r", [128, 92], f32, kind="ExternalInput")
    out_d = nc.dram_tensor("out", [NPC, N], f32, kind="ExternalOutput")

    groups = [list(range(NC_))]

    with tile.TileContext(nc) as tc:
        with (
            tc.tile_pool(name="persist", bufs=1) as pp,
            tc.tile_pool(name="small", bufs=2) as sp,
            tc.tile_pool(name="psum_t", bufs=2, space="PSUM") as pst,
            tc.tile_pool(name="psum_mm", bufs=3, space="PSUM") as pso,
            tc.tile_pool(name="psum_st", bufs=1, space="PSUM") as pss,
            tc.tile_pool(name="dram", bufs=1, space="DRAM") as dp,
        ):
            # ---- internal DRAM (pool tiles => dependency-tracked) ----
            table = dp.tile([TABLE_ROWS, 128], f32)
            hnew_bounce = dp.tile([NPC, H], f32)
            hnew_gath = dp.tile([N, H], f32)
            bn_in = dp.tile([1, 160], f32)
            bn_out = dp.tile([1, 160], f32)
            ct_bounce = dp.tile([34, NPC], f32)
            ct_gath = dp.tile([NC_ * 34, NPC], f32)

            # ---- persistent tiles ----
            idx_sb = pp.tile([128, G * 8], i16)
            nc.sync.dma_start(out=idx_sb[:], in_=idx_d[:])
            consts_sb = pp.tile([128, 5 * NB], f32)
            nc.sync.dma_start(out=consts_sb[:], in_=consts_d[:])
            weff_sb = pp.tile([128, NL * 4 * 240], f32)
            nc.sync.dma_start(out=weff_sb[:], in_=w_eff_d[:])
            beff_sb = pp.tile([128, NL * 3 * 80], f32)
            nc.sync.dma_start(out=beff_sb[:], in_=b_eff_d[:])
            bn_sb = pp.tile([128, 2 * NL * 80], f32)
            nc.sync.dma_start(out=bn_sb[:], in_=bn_d[:])
            ident = pp.tile([128, 128], f32)
            masks.make_identity(nc, ident[:])
            ones_sb = pp.tile([128, 1], f32)
            nc.vector.memset(ones_sb[:], 1.0)
            h_own = pp.tile([128, NB * H], f32)
            o_all = pp.tile([128, NB * H], f32)
            e16 = pp.tile([128, NB * 16], f32)
            zrow = pp.tile([1, 128], f32)
            nc.vector.memset(zrow[:], 0.0)
            c_eps = pp.tile([128, 1], f32)
            nc.vector.memset(c_eps[:], 1e-5)
            c_tiny = pp.tile([128, 1], f32)
            nc.vector.memset(c_tiny[:], 1e-30)
            c_tanhb = pp.tile([128, 1], f32)
            nc.vector.memset(c_tanhb[:], 0.5 * LN_A)

            def cst(k, b):
                return consts_sb[:, k * NB + b:k * NB + b + 1]

            def weff(l, kt):
                o = (l * 4 + kt) * 240
                return weff_sb[:, o:o + 240]

            def beff(l, v):
                o = (l * 3 + v) * 80
                return beff_sb[:, o:o + 80]

            # table zero row
            nc.sync.dma_start(out=table[ZERO_ROW:ZERO_ROW + 1, :], in_=zrow[:])

            # ================= prologue: embed + edge-attr aggregates ======
            with tc.tile_pool(name="prol", bufs=2) as prp, \
                 tc.tile_pool(name="prol1", bufs=1) as pr1:
                wemb_sb = pr1.tile([F_IN, H], f32)
                nc.sync.dma_start(out=wemb_sb[:], in_=w_emb_d[:])
                bemb_sb = pr1.tile([128, H], f32)
                nc.sync.dma_start(out=bemb_sb[:], in_=b_emb_d[:])
                xt_sb = pr1.tile([F_IN, N], f32)
                nc.sync.dma_start(out=xt_sb[:], in_=xt_d[:])
                xto_sb = pr1.tile([F_IN, NPC], f32)
                nc.sync.dma_start(out=xto_sb[:], in_=xt_own_d[:])

                for k in range(N // 128):
                    ps = pso.tile([128, 512], f32, tag="mm")
                    nc.tensor.matmul(ps[:, 0:H], xt_sb[:, k * 128:(k + 1) * 128],
                                     wemb_sb[:], start=True, stop=True)
                    hch = prp.tile([128, 128], f32, tag="hch")
                    nc.vector.tensor_tensor(out=hch[:, 0:H], in0=ps[:, 0:H],
                                            in1=bemb_sb[:], op=OP.add)
                    nc.vector.memset(hch[:, H:128], 0.0)
                    nc.sync.dma_start(out=table[k * 128:(k + 1) * 128, :],
                                      in_=hch[:])
                for b in range(NB):
                    ps = pso.tile([128, 512], f32, tag="mm")
                    nc.tensor.matmul(ps[:, 0:H], xto_sb[:, b * 128:(b + 1) * 128],
                                     wemb_sb[:], start=True, stop=True)
                    nc.vector.tensor_tensor(out=h_own[:, b * H:(b + 1) * H],
                                            in0=ps[:, 0:H], in1=bemb_sb[:],
                                            op=OP.add)

                # edge-attr aggregates -> e16
                ea_sb = pr1.tile([128, G, ED], f32)
                nc.sync.dma_start(out=ea_sb[:],
                                  in_=eattr_d[:].rearrange("p (g e) -> p g e", e=ED))
                sqt = pr1.tile([128, G, ED], f32)
                for b in range(NB):
                    Db, ob = D[b], offs[b]
                    eb = ea_sb[:, ob:ob + Db, :].rearrange("p g e -> p e g")
                    esum = prp.tile([128, ED], f32, tag="esum")
                    emean = prp.tile([128, ED], f32, tag="emean")
                    ess = prp.tile([128, ED], f32, tag="ess")
                    tmp = prp.tile([128, ED], f32, tag="etmp")
                    tmp2 = prp.tile([128, ED], f32, tag="etmp2")
                    nc.vector.tensor_reduce(out=e16[:, b * 16 + 4:b * 16 + 8],
                                            in_=eb, axis=AX.X, op=OP.min)
                    nc.vector.tensor_reduce(out=e16[:, b * 16 + 8:b * 16 + 12],
                                            in_=eb, axis=AX.X, op=OP.max)
                    nc.vector.tensor_reduce(out=esum[:], in_=eb, axis=AX.X,
                                            op=OP.add)
                    e0 = ea_sb[:, ob:ob + 1, :].rearrange("p g e -> p (g e)")
                    nc.vector.tensor_scalar(out=tmp[:], in0=e0,
                                            scalar1=cst(K_NPAD, b), scalar2=None,
                                            op0=OP.mult)
                    nc.vector.tensor_tensor(out=esum[:], in0=esum[:], in1=tmp[:],
                                            op=OP.subtract)
                    nc.vector.tensor_scalar(out=emean[:], in0=esum[:],
                                            scalar1=cst(K_INVDEG, b),
                                            scalar2=None, op0=OP.mult)
                    nc.vector.tensor_copy(out=e16[:, b * 16:b * 16 + 4],
                                          in_=emean[:])
                    nc.scalar.activation(sqt[:, ob:ob + Db, :],
                                         ea_sb[:, ob:ob + Db, :], AF.Square)
                    nc.vector.tensor_reduce(
                        out=ess[:],
                        in_=sqt[:, ob:ob + Db, :].rearrange("p g e -> p e g"),
                        axis=AX.X, op=OP.add)
                    e0sq = sqt[:, ob:ob + 1, :].rearrange("p g e -> p (g e)")
                    nc.vector.tensor_scalar(out=tmp[:], in0=e0sq,
                                            scalar1=cst(K_NPAD, b), scalar2=None,
                                            op0=OP.mult)
                    nc.vector.tensor_tensor(out=ess[:], in0=ess[:], in1=tmp[:],
                                            op=OP.subtract)
                    nc.scalar.activation(tmp2[:], emean[:], AF.Square)
                    nc.vector.scalar_tensor_tensor(out=tmp[:], in0=ess[:],
                                                   scalar=cst(K_INVDEG, b),
                                                   in1=tmp2[:], op0=OP.mult,
                                                   op1=OP.subtract)
                    nc.vector.tensor_scalar_max(out=tmp[:], in0=tmp[:],
                                                scalar1=0.0)
                    nc.scalar.activation(e16[:, b * 16 + 12:b * 16 + 16], tmp[:],
                                         AF.Sqrt, bias=c_eps[:])

            # ================= message-passing layers ======================
            with tc.tile_pool(name="gath", bufs=2) as gp, \
                 tc.tile_pool(name="sqp", bufs=1) as sqp:
                CG = 7  # slot-groups per dma_gather chunk (ring budget)
                for l in range(NL):
                    stats_ps = pss.tile([1, 160], f32, tag="stats")
                    for b in range(NB):
                        Db, ob = D[b], offs[b]
                        Gm = sp.tile([128, 416], f32, tag="Gm")
                        gsum = sp.tile([128, H], f32, tag="gsum")
                        gss = sp.tile([128, H], f32, tag="gss")
                        tmp = sp.tile([128, H], f32, tag="tmp")
                        tmp2 = sp.tile([128, H], f32, tag="tmp2")
                        gt = gp.tile([128, DMAX, 128], f32, tag="gt")
                        for s0 in range(0, Db, CG):
                            cg = min(CG, Db - s0)
                            if NO_GATHER:
                                nc.vector.memset(gt[:, s0:s0 + cg, :], 0.5)
                            else:
                                nc.gpsimd.dma_gather(
                                    gt[:, s0:s0 + cg, :], table[:, :],
                                    idx_sb[:, (ob + s0) * 8:(ob + s0 + cg) * 8],
                                    cg * 128, cg * 128, 128)
                        gv = gt[:, 0:Db, 0:H].rearrange("p g f -> p f g")
                        g0 = gt[:, 0:1, 0:H].rearrange("p g f -> p (g f)")
                        sq = sqp.tile([128, DMAX, H], f32, tag="sq")
                        nc.scalar.activation(sq[:, 0:Db, :], gt[:, 0:Db, 0:H],
                                             AF.Square)
                        sv = sq[:, 0:Db, :].rearrange("p g f -> p f g")
                        nc.vector.tensor_reduce(out=Gm[:, 160:240], in_=gv,
                                                axis=AX.X, op=OP.min)
                        nc.vector.tensor_reduce(out=Gm[:, 240:320], in_=gv,
                                                axis=AX.X, op=OP.max)
                        nc.vector.tensor_reduce(out=gsum[:], in_=gv,
                                                axis=AX.X, op=OP.add)
                        nc.vector.tensor_reduce(out=gss[:], in_=sv,
                                                axis=AX.X, op=OP.add)
                        # corrections: sum/ss -= npad * g0 (, g0^2)
                        nc.vector.tensor_scalar(out=tmp[:], in0=g0,
                                                scalar1=cst(K_NPAD, b),
                                                scalar2=None, op0=OP.mult)
                        nc.vector.tensor_tensor(out=gsum[:], in0=gsum[:],
                                                in1=tmp[:], op=OP.subtract)
                        nc.vector.tensor_scalar(out=Gm[:, 80:160], in0=gsum[:],
                                                scalar1=cst(K_INVDEG, b),
                                                scalar2=None, op0=OP.mult)
                        sq0 = sq[:, 0:1, :].rearrange("p g f -> p (g f)")
                        nc.vector.tensor_scalar(out=tmp[:], in0=sq0,
                                                scalar1=cst(K_NPAD, b),
                                                scalar2=None, op0=OP.mult)
                        nc.vector.tensor_tensor(out=gss[:], in0=gss[:],
                                                in1=tmp[:], op=OP.subtract)
                        nc.scalar.activation(tmp2[:], Gm[:, 80:160], AF.Square)
                        nc.vector.scalar_tensor_tensor(out=tmp[:], in0=gss[:],
                                                       scalar=cst(K_INVDEG, b),
                                                       in1=tmp2[:], op0=OP.mult,
                                                       op1=OP.subtract)
                        nc.vector.tensor_scalar_max(out=tmp[:], in0=tmp[:],
                                                    scalar1=0.0)
                        nc.scalar.activation(Gm[:, 320:400], tmp[:], AF.Sqrt,
                                             bias=c_eps[:])
                        nc.vector.tensor_scalar(out=Gm[:, 0:80],
                                                in0=h_own[:, b * H:(b + 1) * H],
                                                scalar1=cst(K_HAS, b),
                                                scalar2=None, op0=OP.mult)
                        nc.vector.tensor_copy(out=Gm[:, 400:416],
                                              in_=e16[:, b * 16:(b + 1) * 16])
                        ops = pso.tile([128, 512], f32, tag="mm")
                        for kt in range(4):
                            kw = min(128, 416 - kt * 128)
                            pt = pst.tile([128, 128], f32, tag="tr")
                            nc.tensor.transpose(pt[0:kw, :],
                                                Gm[:, kt * 128:kt * 128 + kw],
                                                ident[:])
                            gT = sp.tile([128, 128], f32, tag="gT")
                            nc.vector.tensor_copy(out=gT[0:kw, :],
                                                  in_=pt[0:kw, :])
                            nc.tensor.matmul(ops[:, 0:240], gT[0:kw, :],
                                             weff(l, kt)[0:kw, :],
                                             start=(kt == 0), stop=(kt == 3))
                        ot = o_all[:, b * H:(b + 1) * H]
                        nc.vector.tensor_tensor(out=ot, in0=ops[:, 0:80],
                                                in1=beff(l, 0), op=OP.add)
                        u = sp.tile([128, H], f32, tag="uep")
                        nc.vector.tensor_tensor(out=u[:], in0=ops[:, 80:160],
                                                in1=beff(l, 1), op=OP.add)
                        nc.vector.scalar_tensor_tensor(out=ot, in0=u[:],
                                                       scalar=cst(K_AMP, b),
                                                       in1=ot, op0=OP.mult,
                                                       op1=OP.add)
                        nc.vector.tensor_tensor(out=u[:], in0=ops[:, 160:240],
                                                in1=beff(l, 2), op=OP.add)
                        nc.vector.scalar_tensor_tensor(out=ot, in0=u[:],
                                                       scalar=cst(K_ATT, b),
                                                       in1=ot, op0=OP.mult,
                                                       op1=OP.add)
                        st = sp.tile([128, 160], f32, tag="stin")
                        nc.vector.tensor_copy(out=st[:, 0:80], in_=ot)
                        nc.scalar.activation(st[:, 80:160], ot, AF.Square)
                        nc.tensor.matmul(stats_ps[:], ones_sb[:], st[:],
                                         start=(b == 0), stop=(b == NB - 1))

                    # BN: AllReduce stats, compute scale/shift (replicated)
                    st_sb = sp.tile([1, 160], f32, tag="stsb")
                    nc.vector.tensor_copy(out=st_sb[:], in_=stats_ps[:])
                    nc.sync.dma_start(out=bn_in[:], in_=st_sb[:])
                    if NO_CC:
                        nc.sync.dma_start(out=bn_out[:], in_=bn_in[:])
                    else:
                        nc.gpsimd.collective_compute(
                            "AllReduce", OP.add, replica_groups=groups,
                            ins=[bn_in[:].opt()], outs=[bn_out[:].opt()])
                    mv1 = sp.tile([1, 160], f32, tag="mv1")
                    nc.sync.dma_start(out=mv1[:], in_=bn_out[:])
                    mv = sp.tile([128, 160], f32, tag="mv")
                    if NO_PB:
                        nc.sync.dma_start(
                            out=mv[:],
                            in_=bn_out[:].to_broadcast((128, 160)))
                    else:
                        nc.gpsimd.partition_broadcast(mv[:], mv1[:])
                    mu = sp.tile([128, H], f32, tag="mu")
                    nc.vector.tensor_scalar(out=mu[:], in0=mv[:, 0:80],
                                            scalar1=1.0 / N, scalar2=None,
                                            op0=OP.mult)
                    ex2 = sp.tile([128, H], f32, tag="ex2")
                    nc.vector.tensor_scalar(out=ex2[:], in0=mv[:, 80:160],
                                            scalar1=1.0 / N, scalar2=None,
                                            op0=OP.mult)
                    musq = sp.tile([128, H], f32, tag="musq")
                    nc.scalar.activation(musq[:], mu[:], AF.Square)
                    var = sp.tile([128, H], f32, tag="var")
                    nc.vector.tensor_tensor(out=var[:], in0=ex2[:], in1=musq[:],
                                            op=OP.subtract)
                    nc.scalar.activation(var[:], var[:], AF.Sqrt,
                                         bias=c_eps[:])
                    rinv = sp.tile([128, H], f32, tag="rinv")
                    nc.vector.reciprocal(rinv[:], var[:])
                    grinv = sp.tile([128, H], f32, tag="grinv")
                    nc.vector.tensor_tensor(out=grinv[:], in0=rinv[:],
                                            in1=bn_sb[:, l * 80:(l + 1) * 80],
                                            op=OP.mult)
                    beta = bn_sb[:, (NL + l) * 80:(NL + l + 1) * 80]
                    for b in range(NB):
                        ot = o_all[:, b * H:(b + 1) * H]
                        hb = h_own[:, b * H:(b + 1) * H]
                        t1 = sp.tile([128, H], f32, tag="t1")
                        nc.vector.tensor_tensor(out=t1[:], in0=ot, in1=mu[:],
                                                op=OP.subtract)
                        nc.vector.tensor_tensor(out=t1[:], in0=t1[:],
                                                in1=grinv[:], op=OP.mult)
                        nc.vector.tensor_tensor(out=t1[:], in0=t1[:], in1=beta,
                                                op=OP.add)
                        nc.vector.tensor_scalar_max(out=t1[:], in0=t1[:],
                                                    scalar1=0.0)
                        nc.vector.tensor_tensor(out=hb, in0=t1[:], in1=hb,
                                                op=OP.add)
                        if l < NL - 1:
                            nc.sync.dma_start(
                                out=hnew_bounce[b * 128:(b + 1) * 128, :],
                                in_=hb)
                    if l < NL - 1:
                        if NO_CC:
                            for cc in range(NC_):
                                nc.sync.dma_start(
                                    out=hnew_gath[cc * NPC:(cc + 1) * NPC, :],
                                    in_=hnew_bounce[:])
                        else:
                            nc.gpsimd.collective_compute(
                                "AllGather", OP.bypass, replica_groups=groups,
                                ins=[hnew_bounce[:].opt()],
                                outs=[hnew_gath[:].opt()])
                        nc.sync.dma_start(out=table[0:N, 0:H],
                                          in_=hnew_gath[:])

            # ================= final MLP + distance ========================
            with tc.tile_pool(name="fin", bufs=1) as fp, \
                 tc.tile_pool(name="span", bufs=1) as spp, \
                 tc.tile_pool(name="ospan", bufs=2) as osp:
                w1_sb = fp.tile([80, 40], f32)
                nc.sync.dma_start(out=w1_sb[:], in_=w1_d[:])
                w2_sb = fp.tile([40, 20], f32)
                nc.sync.dma_start(out=w2_sb[:], in_=w2_d[:])
                w3_sb = fp.tile([20, 32], f32)
                nc.sync.dma_start(out=w3_sb[:], in_=w3_d[:])
                mlpb_sb = fp.tile([128, 92], f32)
                nc.sync.dma_start(out=mlpb_sb[:], in_=mlpb_d[:])
                ct_own = fp.tile([34, NPC], f32)
                rt_own = fp.tile([34, NPC], f32)

                for b in range(NB):
                    hb = h_own[:, b * H:(b + 1) * H]
                    ptr = pst.tile([128, 128], f32, tag="tr")
                    nc.tensor.transpose(ptr[0:H, :], hb, ident[:])
                    hT = sp.tile([80, 128], f32, tag="hT")
                    nc.vector.tensor_copy(out=hT[:], in_=ptr[0:H, :])
                    ps1 = pso.tile([128, 512], f32, tag="mm")
                    nc.tensor.matmul(ps1[:, 0:40], hT[:], w1_sb[:],
                                     start=True, stop=True)
                    y1 = sp.tile([128, 40], f32, tag="y1")
                    nc.vector.tensor_tensor(out=y1[:], in0=ps1[:, 0:40],
                                            in1=mlpb_sb[:, 0:40], op=OP.add)
                    nc.vector.tensor_scalar_max(out=y1[:], in0=y1[:],
                                                scalar1=0.0)
                    ptr = pst.tile([128, 128], f32, tag="tr")
                    nc.tensor.transpose(ptr[0:40, :], y1[:], ident[:])
                    y1T = sp.tile([40, 128], f32, tag="y1T")
                    nc.vector.tensor_copy(out=y1T[:], in_=ptr[0:40, :])
                    ps2 = pso.tile([128, 512], f32, tag="mm")
                    nc.tensor.matmul(ps2[:, 0:20], y1T[:], w2_sb[:],
                                     start=True, stop=True)
                    y2 = sp.tile([128, 20], f32, tag="y2")
                    nc.vector.tensor_tensor(out=y2[:], in0=ps2[:, 0:20],
                                            in1=mlpb_sb[:, 40:60], op=OP.add)
                    nc.vector.tensor_scalar_max(out=y2[:], in0=y2[:],
                                                scalar1=0.0)
                    ptr = pst.tile([128, 128], f32, tag="tr")
                    nc.tensor.transpose(ptr[0:20, :], y2[:], ident[:])
                    y2T = sp.tile([20, 128], f32, tag="y2T")
                    nc.vector.tensor_copy(out=y2T[:], in_=ptr[0:20, :])
                    ps3 = pso.tile([128, 512], f32, tag="mm")
                    nc.tensor.matmul(ps3[:, 0:32], y2T[:], w3_sb[:],
                                     start=True, stop=True)
                    y3 = sp.tile([128, 32], f32, tag="y3")
                    nc.vector.tensor_tensor(out=y3[:], in0=ps3[:, 0:32],
                                            in1=mlpb_sb[:, 60:92], op=OP.add)
                    y3sq = sp.tile([128, 32], f32, tag="y3sq")
                    nc.scalar.activation(y3sq[:], y3[:], AF.Square)
                    yext = sp.tile([128, 34], f32, tag="yext")
                    rext = sp.tile([128, 34], f32, tag="rext")
                    nc.vector.tensor_copy(out=yext[:, 0:32], in_=y3[:])
                    nc.vector.tensor_reduce(out=yext[:, 32:33], in_=y3sq[:],
                                            axis=AX.X, op=OP.add)
                    nc.vector.memset(yext[:, 33:34], 1.0)
                    nc.vector.tensor_scalar(out=rext[:, 0:32], in0=y3[:],
                                            scalar1=-2.0, scalar2=None,
                                            op0=OP.mult)
                    nc.vector.memset(rext[:, 32:33], 1.0)
                    nc.vector.tensor_copy(out=rext[:, 33:34],
                                          in_=yext[:, 32:33])
                    ptr = pst.tile([128, 128], f32, tag="tr")
                    nc.tensor.transpose(ptr[0:34, :], yext[:], ident[:])
                    nc.vector.tensor_copy(out=ct_own[:, b * 128:(b + 1) * 128],
                                          in_=ptr[0:34, :])
                    ptr = pst.tile([128, 128], f32, tag="tr")
                    nc.tensor.transpose(ptr[0:34, :], rext[:], ident[:])
                    nc.vector.tensor_copy(out=rt_own[:, b * 128:(b + 1) * 128],
                                          in_=ptr[0:34, :])

                nc.sync.dma_start(out=ct_bounce[:], in_=ct_own[:])
                if NO_CC:
                    for cc in range(NC_):
                        nc.sync.dma_start(
                            out=ct_gath[cc * 34:(cc + 1) * 34, :],
                            in_=ct_bounce[:])
                else:
                    nc.gpsimd.collective_compute(
                        "AllGather", OP.bypass, replica_groups=groups,
                        ins=[ct_bounce[:].opt()], outs=[ct_gath[:].opt()])
                rhs_all = fp.tile([34, N], f32)
                for c in range(NC_):
                    nc.sync.dma_start(out=rhs_all[:, c * NPC:(c + 1) * NPC],
                                      in_=ct_gath[c * 34:(c + 1) * 34, :])

                MB2 = 2  # row-chunks per activation-table-switch phase
                for m0 in range(0, NPC // 128, MB2):
                    spans = []
                    for mi in range(MB2):
                        m = m0 + mi
                        span = spp.tile([128, N], f32, tag=f"span{mi}")
                        spans.append(span)
                        for t in range(N // 512):
                            psd = pso.tile([128, 512], f32, tag="mm")
                            nc.tensor.matmul(psd[:],
                                             rt_own[:, m * 128:(m + 1) * 128],
                                             rhs_all[:, t * 512:(t + 1) * 512],
                                             start=True, stop=True)
                            nc.vector.tensor_scalar_max(
                                out=span[:, t * 512:(t + 1) * 512], in0=psd[:],
                                scalar1=0.0)
                    for mi in range(MB2):
                        nc.scalar.activation(spans[mi][:], spans[mi][:], AF.Ln,
                                             bias=c_tiny[:])
                    for mi in range(MB2):
                        nc.scalar.activation(spans[mi][:], spans[mi][:], AF.Tanh,
                                             scale=0.5 * B_UMAP, bias=c_tanhb[:])
                    for mi in range(MB2):
                        m = m0 + mi
                        for hlf in range(2):
                            sl = slice(hlf * (N // 2), (hlf + 1) * (N // 2))
                            ot = osp.tile([128, N // 2], f32, tag="ospan")
                            nc.vector.tensor_scalar(out=ot[:],
                                                    in0=spans[mi][:, sl],
                                                    scalar1=-0.5, scalar2=0.5,
                                                    op0=OP.mult, op1=OP.add)
                            nc.sync.dma_start(
                                out=out_d[m * 128:(m + 1) * 128, sl],
                                in_=ot[:])

    nc.compile()
    return nc


# --------------------------------------------------------------------------
# entry point
# --------------------------------------------------------------------------
def get_program(D):
    import os
    key = (tuple(D), os.environ.get("NO_GATHER"), os.environ.get("NO_CC"),
           os.environ.get("NO_PB"))
    if key not in _cache:
        _cache[key] = _build(D)
    return _cache[key]


def make_in_maps(inputs):
    shared, per_core, perm_nodes, D = _prepare(inputs)
    in_maps = []
    for c in range(NC_):
        m = dict(shared)
        m.update(per_core[c])
        in_maps.append(m)
    return in_maps, perm_nodes, D


def kernel(**inputs):
    from concourse.bass_utils import run_bass_kernel_spmd

    in_maps, perm_nodes, D = make_in_maps(inputs)
    nc = get_program(D)
    res = run_bass_kernel_spmd(nc, in_maps, list(range(NC_)))
    dev = np.concatenate([res.results[c]["out"] for c in range(NC_)], axis=0)

    out = np.empty((N, N), dtype=np.float32)
    out[np.ix_(perm_nodes, perm_nodes)] = dev
    return out



# revision 19
# speedup vs baseline: 19.4347x; 19.4347x over previous
"""Trainium2 Bass kernel for nn_Net_41807211660013 (PNA-style GNN + UMAP head).

Contract: kernel(**inputs) takes FULL unsharded inputs (as from
reference.setup_inputs()) and returns the FULL [8192, 8192] float32 output.

Strategy (8 NeuronCores, SPMD):
  - nodes sharded by id range: core c owns dst nodes [1024c, 1024(c+1))
  - host reorders each core's nodes by degree into 8 buckets of 128 lanes with
    per-bucket uniform padded degree D_b (pad slots repeat a real neighbor);
    all gather indices are in this permuted order
  - per layer: dma_gather of h rows from an HBM table -> [128, D_b, 128] f32
    tiles feed segmented min/max via DVE tensor_reduce; sum/sumsq come from a
    PE matmul of host-built bf16 adjacency counts against a bf16 [h ; h^2]
    copy of the table (exact counts, so no padding corrections); folded W_post
    matmul on PE; BatchNorm stats via ones-matmul + AllReduce (Shared dram);
    residual; AllGather of the new h shard rebuilds the table
  - final: 3-layer MLP, AllGather of augmented y^T (bf16), row-sharded NxN
    bf16 distance matmul with p = sigmoid(-B*ln(relu(d2)+tiny) - ln(A))
  - host un-permutes output rows/cols
"""
import sys

if "/opt/trn_rl_repo" not in sys.path:
    sys.path.insert(0, "/opt/trn_rl_repo")

import numpy as np

N, E, F_IN, H, ED = 8192, 524288, 39, 80, 4
A_UMAP, B_UMAP = 0.583, 1.334
NC_ = 8
NPC = N // NC_            # 1024 nodes per core
NB = NPC // 128           # 8 buckets
ZERO_ROW = N              # table row of zeros
TABLE_ROWS = N + 1
NL = 4                    # message-passing layers
SQRT_EPS = float(np.sqrt(np.float32(1e-5)))
LN_A = float(np.log(np.float32(A_UMAP)))

# const layout indices (consts tile is [128, 5*NB], slice [:, k*NB+b])
K_INVDEG, K_NPAD, K_AMP, K_ATT, K_HAS = range(5)

_cache = {}


# --------------------------------------------------------------------------
# host preprocessing
# --------------------------------------------------------------------------
def _prepare(inputs):
    x = np.asarray(inputs["x"], np.float32)
    edge_attr = np.asarray(inputs["edge_attr"], np.float32)
    edge_index = np.asarray(inputs["edge_index"], np.int64)
    src_arr, dst_arr = edge_index[0], edge_index[1]

    deg = np.bincount(dst_arr, minlength=N).astype(np.float32)
    logd = np.log1p(deg)
    avg_log = logd.mean(dtype=np.float32)
    amp = (logd / avg_log).astype(np.float32)
    att = np.where(logd > 0, avg_log / np.where(logd > 0, logd, 1.0), 1.0).astype(np.float32)
    has = (deg > 0).astype(np.float32)
    inv_deg = (1.0 / np.where(deg > 0, deg, 1.0)).astype(np.float32)

    order = np.argsort(dst_arr, kind="stable")
    sorted_src = src_arr[order]
    sorted_eid = order
    starts = np.searchsorted(dst_arr[order], np.arange(N))
    ends = np.searchsorted(dst_arr[order], np.arange(N) + 1)

    perm_nodes = np.empty(N, dtype=np.int64)
    for c in range(NC_):
        own = np.arange(c * NPC, (c + 1) * NPC)
        loc = own[np.argsort(-deg[own], kind="stable")]
        perm_nodes[c * NPC:(c + 1) * NPC] = loc
    perm_pos = np.empty(N, dtype=np.int64)
    perm_pos[perm_nodes] = np.arange(N)

    deg_perm = deg[perm_nodes].astype(np.int64).reshape(NC_, NB, 128)
    D = np.maximum(deg_perm.max(axis=(0, 2)), 1).astype(np.int64)  # [NB]
    G = int(D.sum())

    # slot structures per core
    idx_slots = np.full((NC_, G, 128), ZERO_ROW, dtype=np.int32)   # [c][slot][lane]
    eattr_p = np.zeros((NC_, 128, G, ED), dtype=np.float32)        # partition-major
    npad = np.zeros(N, dtype=np.float32)
    offs = np.concatenate([[0], np.cumsum(D)]).astype(np.int64)
    for c in range(NC_):
        for b in range(NB):
            Db, ob = int(D[b]), int(offs[b])
            for p in range(128):
                r = c * NPC + b * 128 + p
                n = perm_nodes[r]
                d = int(deg[n])
                if d == 0:
                    continue
                srcs = perm_pos[sorted_src[starts[n]:ends[n]]]
                eids = sorted_eid[starts[n]:ends[n]]
                idx_slots[c, ob:ob + d, p] = srcs
                idx_slots[c, ob + d:ob + Db, p] = srcs[0]
                ea = edge_attr[eids]
                eattr_p[c, p, ob:ob + d] = ea
                eattr_p[c, p, ob + d:ob + Db] = ea[0]
                npad[r] = Db - d

    # idx in dma_gather wrap layout: value for slot i lives at [i % 16, i // 16],
    # replicated over all 128 partitions
    idx_wrap = np.zeros((NC_, 128, G * 8), dtype=np.int16)
    for c in range(NC_):
        flat = idx_slots[c].reshape(G * 128)             # i = g*128 + lane
        w = flat.reshape(G * 8, 16).T.astype(np.int16)   # [16, G*8]
        idx_wrap[c] = np.tile(w, (8, 1))

    # per-(core, lane, bucket) consts [128, 5*NB]
    consts = np.zeros((NC_, 128, 5 * NB), dtype=np.float32)
    for c in range(NC_):
        rows = perm_nodes[c * NPC:(c + 1) * NPC].reshape(NB, 128)
        trows = np.arange(c * NPC, (c + 1) * NPC).reshape(NB, 128)
        for b in range(NB):
            consts[c, :, K_INVDEG * NB + b] = inv_deg[rows[b]]
            consts[c, :, K_NPAD * NB + b] = npad[trows[b]]
            consts[c, :, K_AMP * NB + b] = amp[rows[b]]
            consts[c, :, K_ATT * NB + b] = att[rows[b]]
            consts[c, :, K_HAS * NB + b] = has[rows[b]]

    # folded W_post weights
    W_post = np.asarray(inputs["W_post"], np.float32)
    b_post = np.asarray(inputs["b_post"], np.float32)
    W_eff = np.zeros((NL, 416, 240), dtype=np.float32)
    b_eff = np.zeros((NL, 3, 80), dtype=np.float32)
    for l in range(NL):
        W = W_post[l]
        for v in range(3):
            o = 656 * v
            W_hown = W[o + 0:o + 80] + W[o + 164:o + 244] + W[o + 328:o + 408]
            W_e = np.concatenate([W[o + 160:o + 164], W[o + 324:o + 328],
                                  W[o + 488:o + 492], W[o + 652:o + 656]], axis=0)
            W_eff[l, :, 80 * v:80 * (v + 1)] = np.concatenate(
                [W_hown, W[o + 80:o + 160], W[o + 244:o + 324],
                 W[o + 408:o + 488], W[o + 572:o + 652], W_e], axis=0)
            b_eff[l, v] = np.float32(SQRT_EPS) * W[o + 492:o + 572].sum(axis=0)
        b_eff[l, 0] += b_post[l]
    # pack for device: [128, NL, 4, 240] (K-chunk partition-major, zero padded)
    w_eff_packed = np.zeros((128, NL, 4, 240), dtype=np.float32)
    for l in range(NL):
        for kt in range(4):
            kw = min(128, 416 - kt * 128)
            w_eff_packed[:kw, l, kt] = W_eff[l, kt * 128:kt * 128 + kw]
    b_eff_r = np.broadcast_to(b_eff.reshape(1, NL * 3 * 80), (128, NL * 3 * 80)).copy()

    bn_gamma = np.asarray(inputs["bn_gamma"], np.float32)
    bn_beta = np.asarray(inputs["bn_beta"], np.float32)
    bn_r = np.broadcast_to(
        np.concatenate([bn_gamma.reshape(NL * 80), bn_beta.reshape(NL * 80)])
        .reshape(1, 2 * NL * 80), (128, 2 * NL * 80)).copy()

    xt = x.T[:, perm_nodes].copy()                 # [39, 8192] table order

    mlp_b_r = np.broadcast_to(
        np.concatenate([np.asarray(inputs["b1"], np.float32),
                        np.asarray(inputs["b2"], np.float32),
                        np.asarray(inputs["b3"], np.float32)]).reshape(1, 92),
        (128, 92)).copy()

    shared = dict(
        xt=np.ascontiguousarray(xt),
        w_emb=np.asarray(inputs["W_emb"], np.float32),
        b_emb_r=np.broadcast_to(np.asarray(inputs["b_emb"], np.float32)
                                .reshape(1, 80), (128, 80)).copy(),
        w_eff=w_eff_packed.reshape(128, NL * 4 * 240),
        b_eff_r=b_eff_r,
        bn_r=bn_r,
        w1=np.asarray(inputs["W1"], np.float32),
        w2=np.asarray(inputs["W2"], np.float32),
        w3=np.asarray(inputs["W3"], np.float32),
        mlp_b_r=mlp_b_r,
    )
    # per-core adjacency counts A[src_perm_row, dst_lane] (for PE sum/sumsq)
    import ml_dtypes
    acnt = np.zeros((NC_, N, NPC), dtype=np.float32)
    src_pos = perm_pos[src_arr]
    dst_pos = perm_pos[dst_arr]
    cores = dst_pos // NPC
    lanes = dst_pos % NPC
    for c in range(NC_):
        m = cores == c
        np.add.at(acnt[c], (src_pos[m], lanes[m]), 1.0)

    per_core = []
    for c in range(NC_):
        per_core.append(dict(
            xt_own=np.ascontiguousarray(xt[:, c * NPC:(c + 1) * NPC]),
            idx=idx_wrap[c],
            eattr=eattr_p[c].reshape(128, G * ED),
            consts=consts[c],
            acnt=acnt[c].astype(ml_dtypes.bfloat16),
        ))
    return shared, per_core, perm_nodes, [int(d) for d in D]


# --------------------------------------------------------------------------
# device program
# --------------------------------------------------------------------------
def _build(D):
    import os
    NO_GATHER = os.environ.get("NO_GATHER") == "1"
    NO_CC = os.environ.get("NO_CC") == "1"
    NO_PB = os.environ.get("NO_PB") == "1"
    CG_ENV = int(os.environ.get("GATHER_CG", "7"))
    PREP = os.environ.get("GATHER_PREP") == "1"
    import concourse.bass as bass  # noqa: F401
    import concourse.bacc as bacc
    import concourse.tile as tile
    import concourse.mybir as mybir
    import concourse.masks as masks

    f32 = mybir.dt.float32
    bf16 = mybir.dt.bfloat16
    i16 = mybir.dt.int16
    AF = mybir.ActivationFunctionType
    OP = mybir.AluOpType
    AX = mybir.AxisListType

    G = sum(D)
    offs = [0]
    for d in D:
        offs.append(offs[-1] + d)
    DMAX = max(D)

    nc = bacc.Bacc("TRN2", target_bir_lowering=False, debug=False,
                   num_devices=NC_)

    # ---- I/O ----
    xt_d = nc.dram_tensor("xt", [F_IN, N], f32, kind="ExternalInput")
    xt_own_d = nc.dram_tensor("xt_own", [F_IN, NPC], f32, kind="ExternalInput")
    w_emb_d = nc.dram_tensor("w_emb", [F_IN, H], f32, kind="ExternalInput")
    b_emb_d = nc.dram_tensor("b_emb_r", [128, H], f32, kind="ExternalInput")
    idx_d = nc.dram_tensor("idx", [128, G * 8], i16, kind="ExternalInput")
    eattr_d = nc.dram_tensor("eattr", [128, G * ED], f32, kind="ExternalInput")
    consts_d = nc.dram_tensor("consts", [128, 5 * NB], f32, kind="ExternalInput")
    w_eff_d = nc.dram_tensor("w_eff", [128, NL * 4 * 240], f32, kind="ExternalInput")
    b_eff_d = nc.dram_tensor("b_eff_r", [128, NL * 3 * 80], f32, kind="ExternalInput")
    bn_d = nc.dram_tensor("bn_r", [128, 2 * NL * 80], f32, kind="ExternalInput")
    w1_d = nc.dram_tensor("w1", [80, 40], f32, kind="ExternalInput")
    w2_d = nc.dram_tensor("w2", [40, 20], f32, kind="ExternalInput")
    w3_d = nc.dram_tensor("w3", [20, 32], f32, kind="ExternalInput")
    mlpb_d = nc.dram_tensor("mlp_b_r", [128, 92], f32, kind="ExternalInput")
    acnt_d = nc.dram_tensor("acnt", [N, NPC], bf16, kind="ExternalInput")
    out_d = nc.dram_tensor("out", [NPC, N], f32, kind="ExternalOutput")

    groups = [list(range(NC_))]

    with tile.TileContext(nc) as tc:
        with (
            tc.tile_pool(name="persist", bufs=1) as pp,
            tc.tile_pool(name="small", bufs=2) as sp,
            tc.tile_pool(name="psum_t", bufs=2, space="PSUM") as pst,
            tc.tile_pool(name="psum_mm", bufs=3, space="PSUM") as pso,
            tc.tile_pool(name="psum_st", bufs=1, space="PSUM") as pss,
            tc.tile_pool(name="dram", bufs=1, space="DRAM") as dp,
        ):
            # ---- internal DRAM (pool tiles => dependency-tracked) ----
            table = dp.tile([TABLE_ROWS, 128], f32)
            hnew_bounce = dp.tile([NPC, H], f32)
            hnew_gaths = [dp.tile([N, H], f32, addr_space="Shared",
                                  name=f"hnew_gath{l}") for l in range(NL - 1)]
            hnew_prol = dp.tile([N, H], f32)
            bn_in = dp.tile([1, 160], f32)
            bn_outs = [dp.tile([1, 160], f32, addr_space="Shared",
                               name=f"bn_out{l}") for l in range(NL)]
            ct_bounce = dp.tile([34, NPC], bf16)
            ct_gath = dp.tile([NC_ * 34, NPC], bf16, addr_space="Shared")

            # ---- persistent tiles ----
            idx_sb = pp.tile([128, G * 8], i16)
            nc.sync.dma_start(out=idx_sb[:], in_=idx_d[:])
            consts_sb = pp.tile([128, 5 * NB], f32)
            nc.sync.dma_start(out=consts_sb[:], in_=consts_d[:])
            weff_sb = pp.tile([128, NL * 4 * 240], f32)
            nc.sync.dma_start(out=weff_sb[:], in_=w_eff_d[:])
            beff_sb = pp.tile([128, NL * 3 * 80], f32)
            nc.sync.dma_start(out=beff_sb[:], in_=b_eff_d[:])
            bn_sb = pp.tile([128, 2 * NL * 80], f32)
            nc.sync.dma_start(out=bn_sb[:], in_=bn_d[:])
            ident = pp.tile([128, 128], f32)
            masks.make_identity(nc, ident[:])
            ones_sb = pp.tile([128, 1], f32)
            nc.vector.memset(ones_sb[:], 1.0)
            h_own = pp.tile([128, NB * H], f32)
            o_all = pp.tile([128, NB * H], f32)
            e16 = pp.tile([128, NB * 16], f32)
            zrow = pp.tile([1, 128], f32)
            nc.vector.memset(zrow[:], 0.0)
            c_eps = pp.tile([128, 1], f32)
            nc.vector.memset(c_eps[:], 1e-5)
            c_tiny = pp.tile([128, 1], f32)
            nc.vector.memset(c_tiny[:], 1e-30)
            c_nlna = pp.tile([128, 1], f32)
            nc.vector.memset(c_nlna[:], -LN_A)

            def cst(k, b):
                return consts_sb[:, k * NB + b:k * NB + b + 1]

            def weff(l, kt):
                o = (l * 4 + kt) * 240
                return weff_sb[:, o:o + 240]

            def beff(l, v):
                o = (l * 3 + v) * 80
                return beff_sb[:, o:o + 80]

            # table zero row
            nc.sync.dma_start(out=table[ZERO_ROW:ZERO_ROW + 1, :], in_=zrow[:])

            # ================= prologue: embed + edge-attr aggregates ======
            with tc.tile_pool(name="prol", bufs=2) as prp, \
                 tc.tile_pool(name="prol1", bufs=1) as pr1:
                wemb_sb = pr1.tile([F_IN, H], f32)
                nc.sync.dma_start(out=wemb_sb[:], in_=w_emb_d[:])
                bemb_sb = pr1.tile([128, H], f32)
                nc.sync.dma_start(out=bemb_sb[:], in_=b_emb_d[:])
                xt_sb = pr1.tile([F_IN, N], f32)
                nc.sync.dma_start(out=xt_sb[:], in_=xt_d[:])
                xto_sb = pr1.tile([F_IN, NPC], f32)
                nc.sync.dma_start(out=xto_sb[:], in_=xt_own_d[:])

                for k in range(N // 128):
                    ps = pso.tile([128, 512], f32, tag="mm")
                    nc.tensor.matmul(ps[:, 0:H], xt_sb[:, k * 128:(k + 1) * 128],
                                     wemb_sb[:], start=True, stop=True)
                    hch = prp.tile([128, 128], f32, tag="hch")
                    nc.vector.tensor_tensor(out=hch[:, 0:H], in0=ps[:, 0:H],
                                            in1=bemb_sb[:], op=OP.add)
                    nc.vector.memset(hch[:, H:128], 0.0)
                    nc.sync.dma_start(out=table[k * 128:(k + 1) * 128, :],
                                      in_=hch[:])
                    nc.sync.dma_start(out=hnew_prol[k * 128:(k + 1) * 128, :],
                                      in_=hch[:, 0:H])
                for b in range(NB):
                    ps = pso.tile([128, 512], f32, tag="mm")
                    nc.tensor.matmul(ps[:, 0:H], xto_sb[:, b * 128:(b + 1) * 128],
                                     wemb_sb[:], start=True, stop=True)
                    nc.vector.tensor_tensor(out=h_own[:, b * H:(b + 1) * H],
                                            in0=ps[:, 0:H], in1=bemb_sb[:],
                                            op=OP.add)

                # edge-attr aggregates -> e16
                ea_sb = pr1.tile([128, G, ED], f32)
                nc.sync.dma_start(out=ea_sb[:],
                                  in_=eattr_d[:].rearrange("p (g e) -> p g e", e=ED))
                sqt = pr1.tile([128, G, ED], f32)
                for b in range(NB):
                    Db, ob = D[b], offs[b]
                    eb = ea_sb[:, ob:ob + Db, :].rearrange("p g e -> p e g")
                    esum = prp.tile([128, ED], f32, tag="esum")
                    emean = prp.tile([128, ED], f32, tag="emean")
                    ess = prp.tile([128, ED], f32, tag="ess")
                    tmp = prp.tile([128, ED], f32, tag="etmp")
                    tmp2 = prp.tile([128, ED], f32, tag="etmp2")
                    nc.vector.tensor_reduce(out=e16[:, b * 16 + 4:b * 16 + 8],
                                            in_=eb, axis=AX.X, op=OP.min)
                    nc.vector.tensor_reduce(out=e16[:, b * 16 + 8:b * 16 + 12],
                                            in_=eb, axis=AX.X, op=OP.max)
                    nc.vector.tensor_reduce(out=esum[:], in_=eb, axis=AX.X,
                                            op=OP.add)
                    e0 = ea_sb[:, ob:ob + 1, :].rearrange("p g e -> p (g e)")
                    nc.vector.tensor_scalar(out=tmp[:], in0=e0,
                                            scalar1=cst(K_NPAD, b), scalar2=None,
                                            op0=OP.mult)
                    nc.vector.tensor_tensor(out=esum[:], in0=esum[:], in1=tmp[:],
                                            op=OP.subtract)
                    nc.vector.tensor_scalar(out=emean[:], in0=esum[:],
                                            scalar1=cst(K_INVDEG, b),
                                            scalar2=None, op0=OP.mult)
                    nc.vector.tensor_copy(out=e16[:, b * 16:b * 16 + 4],
                                          in_=emean[:])
                    nc.scalar.activation(sqt[:, ob:ob + Db, :],
                                         ea_sb[:, ob:ob + Db, :], AF.Square)
                    nc.vector.tensor_reduce(
                        out=ess[:],
                        in_=sqt[:, ob:ob + Db, :].rearrange("p g e -> p e g"),
                        axis=AX.X, op=OP.add)
                    e0sq = sqt[:, ob:ob + 1, :].rearrange("p g e -> p (g e)")
                    nc.vector.tensor_scalar(out=tmp[:], in0=e0sq,
                                            scalar1=cst(K_NPAD, b), scalar2=None,
                                            op0=OP.mult)
                    nc.vector.tensor_tensor(out=ess[:], in0=ess[:], in1=tmp[:],
                                            op=OP.subtract)
                    nc.scalar.activation(tmp2[:], emean[:], AF.Square)
                    nc.vector.scalar_tensor_tensor(out=tmp[:], in0=ess[:],
                                                   scalar=cst(K_INVDEG, b),
                                                   in1=tmp2[:], op0=OP.mult,
                                                   op1=OP.subtract)
                    nc.vector.tensor_scalar_max(out=tmp[:], in0=tmp[:],
                                                scalar1=0.0)
                    nc.scalar.activation(e16[:, b * 16 + 12:b * 16 + 16], tmp[:],
                                         AF.Sqrt, bias=c_eps[:])

            # ================= message-passing layers ======================
            with tc.tile_pool(name="gath", bufs=2) as gp, \
                 tc.tile_pool(name="apool", bufs=1) as apl, \
                 tc.tile_pool(name="tsqp", bufs=1) as tqp, \
                 tc.tile_pool(name="psagg", bufs=2, space="PSUM") as psa:
                CG = CG_ENV  # slot-groups per dma_gather chunk (ring budget)
                gsem = nc.alloc_semaphore("gsem") if PREP else None
                for l in range(NL):
                    stats_ps = pss.tile([1, 160], f32, tag="stats")
                    # bf16 [h ; h^2] table for the A-matmul rhs
                    tsq = tqp.tile([128, 64, 160], bf16, tag="tsq")
                    hl_src = hnew_prol if l == 0 else hnew_gaths[l - 1]
                    for t in range(8):
                        tmpg = sp.tile([128, 8, H], f32, tag="tmpg")
                        nc.sync.dma_start(
                            out=tmpg[:],
                            in_=hl_src[t * 1024:(t + 1) * 1024, :]
                            .rearrange("(kc p) f -> p kc f", p=128))
                        nc.vector.tensor_copy(
                            out=tsq[:, t * 8:(t + 1) * 8, 0:H], in_=tmpg[:])
                        nc.scalar.activation(
                            tsq[:, t * 8:(t + 1) * 8, H:160], tmpg[:],
                            AF.Square)
                    for b in range(NB):
                        Db, ob = D[b], offs[b]
                        Gm = sp.tile([128, 416], f32, tag="Gm")
                        tmp = sp.tile([128, H], f32, tag="tmp")
                        tmp2 = sp.tile([128, H], f32, tag="tmp2")
                        # adjacency counts for this bucket's 128 dst lanes
                        a_sb = apl.tile([128, 64, 128], bf16, tag="a")
                        nc.sync.dma_start(
                            out=a_sb[:],
                            in_=acnt_d[:, b * 128:(b + 1) * 128]
                            .rearrange("(kc p) d -> p kc d", p=128))
                        ps_agg = psa.tile([128, 160], f32, tag="agg")
                        for kc in range(64):
                            nc.tensor.matmul(ps_agg[:], a_sb[:, kc, :],
                                             tsq[:, kc, :],
                                             start=(kc == 0), stop=(kc == 63))
                        gt = gp.tile([128, DMAX, 128], f32, tag="gt")
                        for s0 in range(0, Db, CG):
                            cg = min(CG, Db - s0)
                            if NO_GATHER:
                                nc.vector.memset(gt[:, s0:s0 + cg, :], 0.5)
                            elif PREP:
                                nc.gpsimd.dma_gather(
                                    gt[:, s0:s0 + cg, :], table[:, :],
                                    idx_sb[:, (ob + s0) * 8:(ob + s0 + cg) * 8],
                                    cg * 128, cg * 128, 128,
                                    prepare_only=True, sem=gsem)
                                nc.gpsimd.trigger_dma(count=None)
                            else:
                                nc.gpsimd.dma_gather(
                                    gt[:, s0:s0 + cg, :], table[:, :],
                                    idx_sb[:, (ob + s0) * 8:(ob + s0 + cg) * 8],
                                    cg * 128, cg * 128, 128)
                        gv = gt[:, 0:Db, 0:H].rearrange("p g f -> p f g")
                        nc.vector.tensor_reduce(out=Gm[:, 160:240], in_=gv,
                                                axis=AX.X, op=OP.min)
                        nc.vector.tensor_reduce(out=Gm[:, 240:320], in_=gv,
                                                axis=AX.X, op=OP.max)
                        # mean / std from the A-matmul sums (exact counts)
                        nc.vector.tensor_scalar(out=Gm[:, 80:160],
                                                in0=ps_agg[:, 0:H],
                                                scalar1=cst(K_INVDEG, b),
                                                scalar2=None, op0=OP.mult)
                        nc.scalar.activation(tmp2[:], Gm[:, 80:160], AF.Square)
                        nc.vector.scalar_tensor_tensor(out=tmp[:],
                                                       in0=ps_agg[:, H:160],
                                                       scalar=cst(K_INVDEG, b),
                                                       in1=tmp2[:], op0=OP.mult,
                                                       op1=OP.subtract)
                        nc.vector.tensor_scalar_max(out=tmp[:], in0=tmp[:],
                                                    scalar1=0.0)
                        nc.scalar.activation(Gm[:, 320:400], tmp[:], AF.Sqrt,
                                             bias=c_eps[:])
                        nc.vector.tensor_scalar(out=Gm[:, 0:80],
                                                in0=h_own[:, b * H:(b + 1) * H],
                                                scalar1=cst(K_HAS, b),
                                                scalar2=None, op0=OP.mult)
                        nc.vector.tensor_copy(out=Gm[:, 400:416],
                                              in_=e16[:, b * 16:(b + 1) * 16])
                        ops = pso.tile([128, 512], f32, tag="mm")
                        for kt in range(4):
                            kw = min(128, 416 - kt * 128)
                            pt = pst.tile([128, 128], f32, tag="tr")
                            nc.tensor.transpose(pt[0:kw, :],
                                                Gm[:, kt * 128:kt * 128 + kw],
                                                ident[:])
                            gT = sp.tile([128, 128], f32, tag="gT")
                            nc.vector.tensor_copy(out=gT[0:kw, :],
                                                  in_=pt[0:kw, :])
                            nc.tensor.matmul(ops[:, 0:240], gT[0:kw, :],
                                             weff(l, kt)[0:kw, :],
                                             start=(kt == 0), stop=(kt == 3))
                        ot = o_all[:, b * H:(b + 1) * H]
                        nc.vector.tensor_tensor(out=ot, in0=ops[:, 0:80],
                                                in1=beff(l, 0), op=OP.add)
                        u = sp.tile([128, H], f32, tag="uep")
                        nc.vector.tensor_tensor(out=u[:], in0=ops[:, 80:160],
                                                in1=beff(l, 1), op=OP.add)
                        nc.vector.scalar_tensor_tensor(out=ot, in0=u[:],
                                                       scalar=cst(K_AMP, b),
                                                       in1=ot, op0=OP.mult,
                                                       op1=OP.add)
                        nc.vector.tensor_tensor(out=u[:], in0=ops[:, 160:240],
                                                in1=beff(l, 2), op=OP.add)
                        nc.vector.scalar_tensor_tensor(out=ot, in0=u[:],
                                                       scalar=cst(K_ATT, b),
                                                       in1=ot, op0=OP.mult,
                                                       op1=OP.add)
                        st = sp.tile([128, 160], f32, tag="stin")
                        nc.vector.tensor_copy(out=st[:, 0:80], in_=ot)
                        nc.scalar.activation(st[:, 80:160], ot, AF.Square)
                        nc.tensor.matmul(stats_ps[:], ones_sb[:], st[:],
                                         start=(b == 0), stop=(b == NB - 1))

                    # BN: AllReduce stats, compute scale/shift (replicated)
                    st_sb = sp.tile([1, 160], f32, tag="stsb")
                    nc.vector.tensor_copy(out=st_sb[:], in_=stats_ps[:])
                    nc.sync.dma_start(out=bn_in[:], in_=st_sb[:])
                    bn_out = bn_outs[l]
                    if NO_CC:
                        nc.sync.dma_start(out=bn_out[:], in_=bn_in[:])
                    else:
                        nc.gpsimd.collective_compute(
                            "AllReduce", OP.add, replica_groups=groups,
                            ins=[bn_in[:].opt()], outs=[bn_out[:].opt()])
                    mv1 = sp.tile([1, 160], f32, tag="mv1")
                    nc.sync.dma_start(out=mv1[:], in_=bn_out[:])
                    mv = sp.tile([128, 160], f32, tag="mv")
                    if NO_PB:
                        nc.sync.dma_start(
                            out=mv[:],
                            in_=bn_out[:].to_broadcast((128, 160)))
                    else:
                        nc.gpsimd.partition_broadcast(mv[:], mv1[:])
                    mu = sp.tile([128, H], f32, tag="mu")
                    nc.vector.tensor_scalar(out=mu[:], in0=mv[:, 0:80],
                                            scalar1=1.0 / N, scalar2=None,
                                            op0=OP.mult)
                    ex2 = sp.tile([128, H], f32, tag="ex2")
                    nc.vector.tensor_scalar(out=ex2[:], in0=mv[:, 80:160],
                                            scalar1=1.0 / N, scalar2=None,
                                            op0=OP.mult)
                    musq = sp.tile([128, H], f32, tag="musq")
                    nc.scalar.activation(musq[:], mu[:], AF.Square)
                    var = sp.tile([128, H], f32, tag="var")
                    nc.vector.tensor_tensor(out=var[:], in0=ex2[:], in1=musq[:],
                                            op=OP.subtract)
                    nc.scalar.activation(var[:], var[:], AF.Sqrt,
                                         bias=c_eps[:])
                    rinv = sp.tile([128, H], f32, tag="rinv")
                    nc.vector.reciprocal(rinv[:], var[:])
                    grinv = sp.tile([128, H], f32, tag="grinv")
                    nc.vector.tensor_tensor(out=grinv[:], in0=rinv[:],
                                            in1=bn_sb[:, l * 80:(l + 1) * 80],
                                            op=OP.mult)
                    beta = bn_sb[:, (NL + l) * 80:(NL + l + 1) * 80]
                    for b in range(NB):
                        ot = o_all[:, b * H:(b + 1) * H]
                        hb = h_own[:, b * H:(b + 1) * H]
                        t1 = sp.tile([128, H], f32, tag="t1")
                        nc.vector.tensor_tensor(out=t1[:], in0=ot, in1=mu[:],
                                                op=OP.subtract)
                        nc.vector.tensor_tensor(out=t1[:], in0=t1[:],
                                                in1=grinv[:], op=OP.mult)
                        nc.vector.tensor_tensor(out=t1[:], in0=t1[:], in1=beta,
                                                op=OP.add)
                        nc.vector.tensor_scalar_max(out=t1[:], in0=t1[:],
                                                    scalar1=0.0)
                        nc.vector.tensor_tensor(out=hb, in0=t1[:], in1=hb,
                                                op=OP.add)
                        if l < NL - 1:
                            nc.sync.dma_start(
                                out=hnew_bounce[b * 128:(b + 1) * 128, :],
                                in_=hb)
                    if l < NL - 1:
                        if NO_CC:
                            for cc in range(NC_):
                                nc.sync.dma_start(
                                    out=hnew_gaths[l][cc * NPC:(cc + 1) * NPC, :],
                                    in_=hnew_bounce[:])
                        else:
                            nc.gpsimd.collective_compute(
                                "AllGather", OP.bypass, replica_groups=groups,
                                ins=[hnew_bounce[:].opt()],
                                outs=[hnew_gaths[l][:].opt()])
                        nc.sync.dma_start(out=table[0:N, 0:H],
                                          in_=hnew_gaths[l][:])

            # ================= final MLP + distance ========================
            with tc.tile_pool(name="fin", bufs=1) as fp, \
                 tc.tile_pool(name="span", bufs=1) as spp, \
                 tc.tile_pool(name="ospan", bufs=2) as osp:
                w1_sb = fp.tile([80, 40], f32)
                nc.sync.dma_start(out=w1_sb[:], in_=w1_d[:])
                w2_sb = fp.tile([40, 20], f32)
                nc.sync.dma_start(out=w2_sb[:], in_=w2_d[:])
                w3_sb = fp.tile([20, 32], f32)
                nc.sync.dma_start(out=w3_sb[:], in_=w3_d[:])
                mlpb_sb = fp.tile([128, 92], f32)
                nc.sync.dma_start(out=mlpb_sb[:], in_=mlpb_d[:])
                ct_own = fp.tile([34, NPC], bf16)
                rt_own = fp.tile([34, NPC], bf16)

                for b in range(NB):
                    hb = h_own[:, b * H:(b + 1) * H]
                    ptr = pst.tile([128, 128], f32, tag="tr")
                    nc.tensor.transpose(ptr[0:H, :], hb, ident[:])
                    hT = sp.tile([80, 128], f32, tag="hT")
                    nc.vector.tensor_copy(out=hT[:], in_=ptr[0:H, :])
                    ps1 = pso.tile([128, 512], f32, tag="mm")
                    nc.tensor.matmul(ps1[:, 0:40], hT[:], w1_sb[:],
                                     start=True, stop=True)
                    y1 = sp.tile([128, 40], f32, tag="y1")
                    nc.vector.tensor_tensor(out=y1[:], in0=ps1[:, 0:40],
                                            in1=mlpb_sb[:, 0:40], op=OP.add)
                    nc.vector.tensor_scalar_max(out=y1[:], in0=y1[:],
                                                scalar1=0.0)
                    ptr = pst.tile([128, 128], f32, tag="tr")
                    nc.tensor.transpose(ptr[0:40, :], y1[:], ident[:])
                    y1T = sp.tile([40, 128], f32, tag="y1T")
                    nc.vector.tensor_copy(out=y1T[:], in_=ptr[0:40, :])
                    ps2 = pso.tile([128, 512], f32, tag="mm")
                    nc.tensor.matmul(ps2[:, 0:20], y1T[:], w2_sb[:],
                                     start=True, stop=True)
                    y2 = sp.tile([128, 20], f32, tag="y2")
                    nc.vector.tensor_tensor(out=y2[:], in0=ps2[:, 0:20],
                                            in1=mlpb_sb[:, 40:60], op=OP.add)
                    nc.vector.tensor_scalar_max(out=y2[:], in0=y2[:],
                                                scalar1=0.0)
                    ptr = pst.tile([128, 128], f32, tag="tr")
                    nc.tensor.transpose(ptr[0:20, :], y2[:], ident[:])
                    y2T = sp.tile([20, 128], f32, tag="y2T")
                    nc.vector.tensor_copy(out=y2T[:], in_=ptr[0:20, :])
                    ps3 = pso.tile([128, 512], f32, tag="mm")
                    nc.tensor.matmul(ps3[:, 0:32], y2T[:], w3_sb[:],
                                     start=True, stop=True)
                    y3 = sp.tile([128, 32], f32, tag="y3")
                    nc.vector.tensor_tensor(out=y3[:], in0=ps3[:, 0:32],
                                            in1=mlpb_sb[:, 60:92], op=OP.add)
                    y3sq = sp.tile([128, 32], f32, tag="y3sq")
                    nc.scalar.activation(y3sq[:], y3[:], AF.Square)
                    yext = sp.tile([128, 34], f32, tag="yext")
                    rext = sp.tile([128, 34], f32, tag="rext")
                    nc.vector.tensor_copy(out=yext[:, 0:32], in_=y3[:])
                    nc.vector.tensor_reduce(out=yext[:, 32:33], in_=y3sq[:],
                                            axis=AX.X, op=OP.add)
                    nc.vector.memset(yext[:, 33:34], 1.0)
                    nc.vector.tensor_scalar(out=rext[:, 0:32], in0=y3[:],
                                            scalar1=-2.0, scalar2=None,
                                            op0=OP.mult)
                    nc.vector.memset(rext[:, 32:33], 1.0)
                    nc.vector.tensor_copy(out=rext[:, 33:34],
                                          in_=yext[:, 32:33])
                    ptr = pst.tile([128, 128], f32, tag="tr")
                    nc.tensor.transpose(ptr[0:34, :], yext[:], ident[:])
                    nc.vector.tensor_copy(out=ct_own[:, b * 128:(b + 1) * 128],
                                          in_=ptr[0:34, :])
                    ptr = pst.tile([128, 128], f32, tag="tr")
                    nc.tensor.transpose(ptr[0:34, :], rext[:], ident[:])
                    nc.vector.tensor_copy(out=rt_own[:, b * 128:(b + 1) * 128],
                                          in_=ptr[0:34, :])

                nc.sync.dma_start(out=ct_bounce[:], in_=ct_own[:])
                if NO_CC:
                    for cc in range(NC_):
                        nc.sync.dma_start(
                            out=ct_gath[cc * 34:(cc + 1) * 34, :],
                            in_=ct_bounce[:])
                else:
                    nc.gpsimd.collective_compute(
                        "AllGather", OP.bypass, replica_groups=groups,
                        ins=[ct_bounce[:].opt()], outs=[ct_gath[:].opt()])
                rhs_all = fp.tile([34, N], bf16)
                for c in range(NC_):
                    nc.sync.dma_start(out=rhs_all[:, c * NPC:(c + 1) * NPC],
                                      in_=ct_gath[c * 34:(c + 1) * 34, :])

                MB2 = 2  # row-chunks per activation-table-switch phase
                for m0 in range(0, NPC // 128, MB2):
                    spans = []
                    for mi in range(MB2):
                        m = m0 + mi
                        span = spp.tile([128, N], bf16, tag=f"span{mi}")
                        spans.append(span)
                        for t in range(N // 512):
                            psd = pso.tile([128, 512], f32, tag="mm")
                            nc.tensor.matmul(psd[:],
                                             rt_own[:, m * 128:(m + 1) * 128],
                                             rhs_all[:, t * 512:(t + 1) * 512],
                                             start=True, stop=True)
                            nc.vector.tensor_scalar_max(
                                out=span[:, t * 512:(t + 1) * 512], in0=psd[:],
                                scalar1=0.0)
                    for mi in range(MB2):
                        nc.scalar.activation(spans[mi][:], spans[mi][:], AF.Ln,
                                             bias=c_tiny[:])
                    for mi in range(MB2):
                        m = m0 + mi
                        for hlf in range(2):
                            sl = slice(hlf * (N // 2), (hlf + 1) * (N // 2))
                            ot = osp.tile([128, N // 2], f32, tag="ospan")
                            nc.scalar.activation(ot[:], spans[mi][:, sl],
                                                 AF.Sigmoid, scale=-B_UMAP,
                                                 bias=c_nlna[:])
                            nc.sync.dma_start(
                                out=out_d[m * 128:(m + 1) * 128, sl],
                                in_=ot[:])

    nc.compile()
    return nc


# --------------------------------------------------------------------------
# entry point
# --------------------------------------------------------------------------
def get_program(D):
    import os
    key = (tuple(D), os.environ.get("NO_GATHER"), os.environ.get("NO_CC"),
           os.environ.get("NO_PB"), os.environ.get("GATHER_CG"),
           os.environ.get("GATHER_PREP"))
    if key not in _cache:
        _cache[key] = _build(D)
    return _cache[key]


def make_in_maps(inputs):
    shared, per_core, perm_nodes, D = _prepare(inputs)
    in_maps = []
    for c in range(NC_):
        m = dict(shared)
        m.update(per_core[c])
        in_maps.append(m)
    return in_maps, perm_nodes, D


def kernel(**inputs):
    from concourse.bass_utils import run_bass_kernel_spmd

    in_maps, perm_nodes, D = make_in_maps(inputs)
    nc = get_program(D)
    res = run_bass_kernel_spmd(nc, in_maps, list(range(NC_)))
    dev = np.concatenate([res.results[c]["out"] for c in range(NC_)], axis=0)

    out = np.empty((N, N), dtype=np.float32)
    out[np.ix_(perm_nodes, perm_nodes)] = dev
    return out



# revision 29
# speedup vs baseline: 20.5921x; 1.0596x over previous
"""Trainium2 Bass kernel for nn_Net_41807211660013 (PNA-style GNN + UMAP head).

Contract: kernel(**inputs) takes FULL unsharded inputs (as from
reference.setup_inputs()) and returns the FULL [8192, 8192] float32 output.

Strategy (8 NeuronCores, SPMD):
  - nodes sharded by id range: core c owns dst nodes [1024c, 1024(c+1))
  - host reorders each core's nodes by degree into 8 buckets of 128 lanes with
    per-bucket uniform padded degree D_b (pad slots repeat a real neighbor);
    all gather indices are in this permuted order
  - per layer: dma_gather of h rows from an HBM table -> [128, D_b, 128] f32
    tiles feed segmented min/max via DVE tensor_reduce; sum/sumsq come from a
    PE matmul of host-built bf16 adjacency counts against a bf16 [h ; h^2]
    copy of the table (exact counts, so no padding corrections); folded W_post
    matmul on PE; BatchNorm stats via ones-matmul + AllReduce (Shared dram);
    residual; AllGather of the new h shard rebuilds the table
  - final: 3-layer MLP, AllGather of augmented y^T (bf16), row-sharded NxN
    bf16 distance matmul with p = sigmoid(-B*ln(relu(d2)+tiny) - ln(A))
  - host un-permutes output rows/cols
"""
import sys

if "/opt/trn_rl_repo" not in sys.path:
    sys.path.insert(0, "/opt/trn_rl_repo")

import numpy as np

N, E, F_IN, H, ED = 8192, 524288, 39, 80, 4
A_UMAP, B_UMAP = 0.583, 1.334
NC_ = 8
NPC = N // NC_            # 1024 nodes per core
NB = NPC // 128           # 8 buckets
ZERO_ROW = N              # table row of zeros
TABLE_ROWS = N + 1
NL = 4                    # message-passing layers
SQRT_EPS = float(np.sqrt(np.float32(1e-5)))
LN_A = float(np.log(np.float32(A_UMAP)))

# const layout indices (consts tile is [128, 5*NB], slice [:, k*NB+b])
K_INVDEG, K_NPAD, K_AMP, K_ATT, K_HAS = range(5)

_cache = {}


# --------------------------------------------------------------------------
# host preprocessing
# --------------------------------------------------------------------------
def _prepare(inputs):
    x = np.asarray(inputs["x"], np.float32)
    edge_attr = np.asarray(inputs["edge_attr"], np.float32)
    edge_index = np.asarray(inputs["edge_index"], np.int64)
    src_arr, dst_arr = edge_index[0], edge_index[1]

    deg = np.bincount(dst_arr, minlength=N).astype(np.float32)
    logd = np.log1p(deg)
    avg_log = logd.mean(dtype=np.float32)
    amp = (logd / avg_log).astype(np.float32)
    att = np.where(logd > 0, avg_log / np.where(logd > 0, logd, 1.0), 1.0).astype(np.float32)
    has = (deg > 0).astype(np.float32)
    inv_deg = (1.0 / np.where(deg > 0, deg, 1.0)).astype(np.float32)

    order = np.argsort(dst_arr, kind="stable")
    sorted_src = src_arr[order]
    sorted_eid = order
    starts = np.searchsorted(dst_arr[order], np.arange(N))
    ends = np.searchsorted(dst_arr[order], np.arange(N) + 1)

    perm_nodes = np.empty(N, dtype=np.int64)
    for c in range(NC_):
        own = np.arange(c * NPC, (c + 1) * NPC)
        loc = own[np.argsort(-deg[own], kind="stable")]
        perm_nodes[c * NPC:(c + 1) * NPC] = loc
    perm_pos = np.empty(N, dtype=np.int64)
    perm_pos[perm_nodes] = np.arange(N)

    deg_perm = deg[perm_nodes].astype(np.int64).reshape(NC_, NB, 128)
    D = np.maximum(deg_perm.max(axis=(0, 2)), 1).astype(np.int64)  # [NB]
    G = int(D.sum())

    # slot structures per core
    idx_slots = np.full((NC_, G, 128), ZERO_ROW, dtype=np.int32)   # [c][slot][lane]
    eattr_p = np.zeros((NC_, 128, G, ED), dtype=np.float32)        # partition-major
    npad = np.zeros(N, dtype=np.float32)
    offs = np.concatenate([[0], np.cumsum(D)]).astype(np.int64)
    for c in range(NC_):
        for b in range(NB):
            Db, ob = int(D[b]), int(offs[b])
            for p in range(128):
                r = c * NPC + b * 128 + p
                n = perm_nodes[r]
                d = int(deg[n])
                if d == 0:
                    continue
                srcs = perm_pos[sorted_src[starts[n]:ends[n]]]
                eids = sorted_eid[starts[n]:ends[n]]
                idx_slots[c, ob:ob + d, p] = srcs
                idx_slots[c, ob + d:ob + Db, p] = srcs[0]
                ea = edge_attr[eids]
                eattr_p[c, p, ob:ob + d] = ea
                eattr_p[c, p, ob + d:ob + Db] = ea[0]
                npad[r] = Db - d

    # idx in dma_gather wrap layout: value for slot i lives at [i % 16, i // 16],
    # replicated over all 128 partitions
    idx_wrap = np.zeros((NC_, 128, G * 8), dtype=np.int16)
    for c in range(NC_):
        flat = idx_slots[c].reshape(G * 128)             # i = g*128 + lane
        w = flat.reshape(G * 8, 16).T.astype(np.int16)   # [16, G*8]
        idx_wrap[c] = np.tile(w, (8, 1))

    # per-(core, lane, bucket) consts [128, 5*NB]
    consts = np.zeros((NC_, 128, 5 * NB), dtype=np.float32)
    for c in range(NC_):
        rows = perm_nodes[c * NPC:(c + 1) * NPC].reshape(NB, 128)
        trows = np.arange(c * NPC, (c + 1) * NPC).reshape(NB, 128)
        for b in range(NB):
            consts[c, :, K_INVDEG * NB + b] = inv_deg[rows[b]]
            consts[c, :, K_NPAD * NB + b] = npad[trows[b]]
            consts[c, :, K_AMP * NB + b] = amp[rows[b]]
            consts[c, :, K_ATT * NB + b] = att[rows[b]]
            consts[c, :, K_HAS * NB + b] = has[rows[b]]

    # folded W_post weights
    W_post = np.asarray(inputs["W_post"], np.float32)
    b_post = np.asarray(inputs["b_post"], np.float32)
    W_eff = np.zeros((NL, 416, 240), dtype=np.float32)
    b_eff = np.zeros((NL, 3, 80), dtype=np.float32)
    for l in range(NL):
        W = W_post[l]
        for v in range(3):
            o = 656 * v
            W_hown = W[o + 0:o + 80] + W[o + 164:o + 244] + W[o + 328:o + 408]
            W_e = np.concatenate([W[o + 160:o + 164], W[o + 324:o + 328],
                                  W[o + 488:o + 492], W[o + 652:o + 656]], axis=0)
            W_eff[l, :, 80 * v:80 * (v + 1)] = np.concatenate(
                [W_hown, W[o + 80:o + 160], W[o + 244:o + 324],
                 W[o + 408:o + 488], W[o + 572:o + 652], W_e], axis=0)
            b_eff[l, v] = np.float32(SQRT_EPS) * W[o + 492:o + 572].sum(axis=0)
        b_eff[l, 0] += b_post[l]
    # pack for device: [128, NL, 4, 240] (K-chunk partition-major, zero padded)
    w_eff_packed = np.zeros((128, NL, 4, 240), dtype=np.float32)
    for l in range(NL):
        for kt in range(4):
            kw = min(128, 416 - kt * 128)
            w_eff_packed[:kw, l, kt] = W_eff[l, kt * 128:kt * 128 + kw]
    b_eff_r = np.broadcast_to(b_eff.reshape(1, NL * 3 * 80), (128, NL * 3 * 80)).copy()

    bn_gamma = np.asarray(inputs["bn_gamma"], np.float32)
    bn_beta = np.asarray(inputs["bn_beta"], np.float32)
    bn_r = np.broadcast_to(
        np.concatenate([bn_gamma.reshape(NL * 80), bn_beta.reshape(NL * 80)])
        .reshape(1, 2 * NL * 80), (128, 2 * NL * 80)).copy()

    xt = x.T[:, perm_nodes].copy()                 # [39, 8192] table order

    mlp_b_r = np.broadcast_to(
        np.concatenate([np.asarray(inputs["b1"], np.float32),
                        np.asarray(inputs["b2"], np.float32),
                        np.asarray(inputs["b3"], np.float32)]).reshape(1, 92),
        (128, 92)).copy()

    import ml_dtypes
    shared = dict(
        xt=np.ascontiguousarray(xt).astype(ml_dtypes.bfloat16),
        w_emb=np.asarray(inputs["W_emb"], np.float32).astype(ml_dtypes.bfloat16),
        b_emb_r=np.broadcast_to(np.asarray(inputs["b_emb"], np.float32)
                                .reshape(1, 80), (128, 80)).copy(),
        w_eff=w_eff_packed.reshape(128, NL * 4 * 240),
        b_eff_r=b_eff_r,
        bn_r=bn_r,
        w1=np.asarray(inputs["W1"], np.float32),
        w2=np.asarray(inputs["W2"], np.float32),
        w3=np.asarray(inputs["W3"], np.float32),
        mlp_b_r=mlp_b_r,
    )
    # per-core adjacency counts A[src_perm_row, dst_lane] (for PE sum/sumsq)
    import ml_dtypes
    acnt = np.zeros((NC_, N, NPC), dtype=np.float32)
    src_pos = perm_pos[src_arr]
    dst_pos = perm_pos[dst_arr]
    cores = dst_pos // NPC
    lanes = dst_pos % NPC
    for c in range(NC_):
        m = cores == c
        np.add.at(acnt[c], (src_pos[m], lanes[m]), 1.0)

    per_core = []
    for c in range(NC_):
        per_core.append(dict(
            xt_own=np.ascontiguousarray(xt[:, c * NPC:(c + 1) * NPC])
            .astype(ml_dtypes.bfloat16),
            idx=idx_wrap[c],
            eattr=eattr_p[c].reshape(128, G * ED),
            consts=consts[c],
            acnt=acnt[c].astype(ml_dtypes.bfloat16),
        ))
    return shared, per_core, perm_nodes, [int(d) for d in D]


# --------------------------------------------------------------------------
# device program
# --------------------------------------------------------------------------
def _build(D):
    import os
    NO_GATHER = os.environ.get("NO_GATHER") == "1"
    NO_CC = os.environ.get("NO_CC") == "1"
    NO_PB = os.environ.get("NO_PB") == "1"
    CG_ENV = int(os.environ.get("GATHER_CG", "7"))
    PREP = os.environ.get("GATHER_PREP") == "1"
    import concourse.bass as bass  # noqa: F401
    import concourse.bacc as bacc
    import concourse.tile as tile
    import concourse.mybir as mybir
    import concourse.masks as masks

    f32 = mybir.dt.float32
    bf16 = mybir.dt.bfloat16
    i16 = mybir.dt.int16
    AF = mybir.ActivationFunctionType
    OP = mybir.AluOpType
    AX = mybir.AxisListType

    G = sum(D)
    offs = [0]
    for d in D:
        offs.append(offs[-1] + d)
    DMAX = max(D)

    nc = bacc.Bacc("TRN2", target_bir_lowering=False, debug=False,
                   num_devices=NC_)

    # ---- I/O ----
    xt_d = nc.dram_tensor("xt", [F_IN, N], bf16, kind="ExternalInput")
    xt_own_d = nc.dram_tensor("xt_own", [F_IN, NPC], bf16, kind="ExternalInput")
    w_emb_d = nc.dram_tensor("w_emb", [F_IN, H], bf16, kind="ExternalInput")
    b_emb_d = nc.dram_tensor("b_emb_r", [128, H], f32, kind="ExternalInput")
    idx_d = nc.dram_tensor("idx", [128, G * 8], i16, kind="ExternalInput")
    eattr_d = nc.dram_tensor("eattr", [128, G * ED], f32, kind="ExternalInput")
    consts_d = nc.dram_tensor("consts", [128, 5 * NB], f32, kind="ExternalInput")
    w_eff_d = nc.dram_tensor("w_eff", [128, NL * 4 * 240], f32, kind="ExternalInput")
    b_eff_d = nc.dram_tensor("b_eff_r", [128, NL * 3 * 80], f32, kind="ExternalInput")
    bn_d = nc.dram_tensor("bn_r", [128, 2 * NL * 80], f32, kind="ExternalInput")
    w1_d = nc.dram_tensor("w1", [80, 40], f32, kind="ExternalInput")
    w2_d = nc.dram_tensor("w2", [40, 20], f32, kind="ExternalInput")
    w3_d = nc.dram_tensor("w3", [20, 32], f32, kind="ExternalInput")
    mlpb_d = nc.dram_tensor("mlp_b_r", [128, 92], f32, kind="ExternalInput")
    acnt_d = nc.dram_tensor("acnt", [N, NPC], bf16, kind="ExternalInput")
    out_d = nc.dram_tensor("out", [NPC, N], f32, kind="ExternalOutput")

    groups = [list(range(NC_))]

    with tile.TileContext(nc) as tc:
        with (
            tc.tile_pool(name="persist", bufs=1) as pp,
            tc.tile_pool(name="small", bufs=2) as sp,
            tc.tile_pool(name="psum_t", bufs=2, space="PSUM") as pst,
            tc.tile_pool(name="psum_mm", bufs=3, space="PSUM") as pso,
            tc.tile_pool(name="psum_st", bufs=1, space="PSUM") as pss,
            tc.tile_pool(name="dram", bufs=1, space="DRAM") as dp,
        ):
            # ---- internal DRAM (pool tiles => dependency-tracked) ----
            table = dp.tile([TABLE_ROWS, 128], bf16)
            hnew_bounce = dp.tile([NPC, 128], bf16)
            bn_in = dp.tile([1, 160], f32)
            bn_outs = [dp.tile([1, 160], f32, addr_space="Shared",
                               name=f"bn_out{l}") for l in range(NL)]
            ct_bounce = dp.tile([34, NPC], bf16)
            ct_gath = dp.tile([NC_ * 34, NPC], bf16, addr_space="Shared")

            # ---- persistent tiles ----
            idx_sb = pp.tile([128, G * 8], i16)
            nc.sync.dma_start(out=idx_sb[:], in_=idx_d[:])
            consts_sb = pp.tile([128, 5 * NB], f32)
            nc.sync.dma_start(out=consts_sb[:], in_=consts_d[:])
            weff_sb = pp.tile([128, NL * 4 * 240], f32)
            nc.sync.dma_start(out=weff_sb[:], in_=w_eff_d[:])
            beff_sb = pp.tile([128, NL * 3 * 80], f32)
            nc.sync.dma_start(out=beff_sb[:], in_=b_eff_d[:])
            bn_sb = pp.tile([128, 2 * NL * 80], f32)
            nc.sync.dma_start(out=bn_sb[:], in_=bn_d[:])
            ident = pp.tile([128, 128], f32)
            masks.make_identity(nc, ident[:])
            ones_sb = pp.tile([128, 1], f32)
            nc.vector.memset(ones_sb[:], 1.0)
            h_own = pp.tile([128, NB * H], f32)
            o_all = pp.tile([128, NB * H], f32)
            e16 = pp.tile([128, NB * 16], f32)
            zrow = pp.tile([1, 128], bf16)
            nc.vector.memset(zrow[:], 0.0)
            c_eps = pp.tile([128, 1], f32)
            nc.vector.memset(c_eps[:], 1e-5)
            c_tiny = pp.tile([128, 1], f32)
            nc.vector.memset(c_tiny[:], 1e-30)
            c_nlna = pp.tile([128, 1], f32)
            nc.vector.memset(c_nlna[:], -LN_A)
            w1_sb = pp.tile([80, 40], f32)
            nc.sync.dma_start(out=w1_sb[:], in_=w1_d[:])
            w2_sb = pp.tile([40, 20], f32)
            nc.sync.dma_start(out=w2_sb[:], in_=w2_d[:])
            w3_sb = pp.tile([20, 32], f32)
            nc.sync.dma_start(out=w3_sb[:], in_=w3_d[:])
            mlpb_sb = pp.tile([128, 92], f32)
            nc.sync.dma_start(out=mlpb_sb[:], in_=mlpb_d[:])
            ct_own = pp.tile([34, NPC], bf16)
            rt_own = pp.tile([34, NPC], bf16)

            def mlp_bucket(b):
                hb = h_own[:, b * H:(b + 1) * H]
                ptr = pst.tile([128, 128], f32, tag="tr")
                nc.tensor.transpose(ptr[0:H, :], hb, ident[:])
                hT = sp.tile([80, 128], f32, tag="hT")
                nc.vector.tensor_copy(out=hT[:], in_=ptr[0:H, :])
                ps1 = pso.tile([128, 512], f32, tag="mm")
                nc.tensor.matmul(ps1[:, 0:40], hT[:], w1_sb[:],
                                 start=True, stop=True)
                y1 = sp.tile([128, 40], f32, tag="y1")
                nc.vector.tensor_tensor(out=y1[:], in0=ps1[:, 0:40],
                                        in1=mlpb_sb[:, 0:40], op=OP.add)
                nc.vector.tensor_scalar_max(out=y1[:], in0=y1[:], scalar1=0.0)
                ptr = pst.tile([128, 128], f32, tag="tr")
                nc.tensor.transpose(ptr[0:40, :], y1[:], ident[:])
                y1T = sp.tile([40, 128], f32, tag="y1T")
                nc.vector.tensor_copy(out=y1T[:], in_=ptr[0:40, :])
                ps2 = pso.tile([128, 512], f32, tag="mm")
                nc.tensor.matmul(ps2[:, 0:20], y1T[:], w2_sb[:],
                                 start=True, stop=True)
                y2 = sp.tile([128, 20], f32, tag="y2")
                nc.vector.tensor_tensor(out=y2[:], in0=ps2[:, 0:20],
                                        in1=mlpb_sb[:, 40:60], op=OP.add)
                nc.vector.tensor_scalar_max(out=y2[:], in0=y2[:], scalar1=0.0)
                ptr = pst.tile([128, 128], f32, tag="tr")
                nc.tensor.transpose(ptr[0:20, :], y2[:], ident[:])
                y2T = sp.tile([20, 128], f32, tag="y2T")
                nc.vector.tensor_copy(out=y2T[:], in_=ptr[0:20, :])
                ps3 = pso.tile([128, 512], f32, tag="mm")
                nc.tensor.matmul(ps3[:, 0:32], y2T[:], w3_sb[:],
                                 start=True, stop=True)
                y3 = sp.tile([128, 32], f32, tag="y3")
                nc.vector.tensor_tensor(out=y3[:], in0=ps3[:, 0:32],
                                        in1=mlpb_sb[:, 60:92], op=OP.add)
                y3sq = sp.tile([128, 32], f32, tag="y3sq")
                nc.scalar.activation(y3sq[:], y3[:], AF.Square)
                yext = sp.tile([128, 34], f32, tag="yext")
                rext = sp.tile([128, 34], f32, tag="rext")
                nc.vector.tensor_copy(out=yext[:, 0:32], in_=y3[:])
                nc.vector.tensor_reduce(out=yext[:, 32:33], in_=y3sq[:],
                                        axis=AX.X, op=OP.add)
                nc.vector.memset(yext[:, 33:34], 1.0)
                nc.vector.tensor_scalar(out=rext[:, 0:32], in0=y3[:],
                                        scalar1=-2.0, scalar2=None,
                                        op0=OP.mult)
                nc.vector.memset(rext[:, 32:33], 1.0)
                nc.vector.tensor_copy(out=rext[:, 33:34], in_=yext[:, 32:33])
                ptr = pst.tile([128, 128], f32, tag="tr")
                nc.tensor.transpose(ptr[0:34, :], yext[:], ident[:])
                nc.vector.tensor_copy(out=ct_own[:, b * 128:(b + 1) * 128],
                                      in_=ptr[0:34, :])
                ptr = pst.tile([128, 128], f32, tag="tr")
                nc.tensor.transpose(ptr[0:34, :], rext[:], ident[:])
                nc.vector.tensor_copy(out=rt_own[:, b * 128:(b + 1) * 128],
                                      in_=ptr[0:34, :])

            def cst(k, b):
                return consts_sb[:, k * NB + b:k * NB + b + 1]

            def weff(l, kt):
                o = (l * 4 + kt) * 240
                return weff_sb[:, o:o + 240]

            def beff(l, v):
                o = (l * 3 + v) * 80
                return beff_sb[:, o:o + 80]

            # table zero row
            nc.sync.dma_start(out=table[ZERO_ROW:ZERO_ROW + 1, :], in_=zrow[:])

            # ================= prologue: embed + edge-attr aggregates ======
            with tc.tile_pool(name="prol", bufs=2) as prp, \
                 tc.tile_pool(name="prol1", bufs=1) as pr1:
                wemb_sb = pr1.tile([F_IN, H], bf16)
                nc.sync.dma_start(out=wemb_sb[:], in_=w_emb_d[:])
                bemb_sb = pr1.tile([128, H], f32)
                nc.sync.dma_start(out=bemb_sb[:], in_=b_emb_d[:])
                xt_sb = pr1.tile([F_IN, N], bf16)
                nc.sync.dma_start(out=xt_sb[:], in_=xt_d[:])
                xto_sb = pr1.tile([F_IN, NPC], bf16)
                nc.sync.dma_start(out=xto_sb[:], in_=xt_own_d[:])

                for k in range(N // 128):
                    ps = pso.tile([128, 512], f32, tag="mm")
                    nc.tensor.matmul(ps[:, 0:H], xt_sb[:, k * 128:(k + 1) * 128],
                                     wemb_sb[:], start=True, stop=True)
                    hch = prp.tile([128, H], bf16, tag="hch")
                    nc.vector.tensor_tensor(out=hch[:], in0=ps[:, 0:H],
                                            in1=bemb_sb[:], op=OP.add)
                    nc.sync.dma_start(out=table[k * 128:(k + 1) * 128, 0:H],
                                      in_=hch[:])
                for b in range(NB):
                    ps = pso.tile([128, 512], f32, tag="mm")
                    nc.tensor.matmul(ps[:, 0:H], xto_sb[:, b * 128:(b + 1) * 128],
                                     wemb_sb[:], start=True, stop=True)
                    nc.vector.tensor_tensor(out=h_own[:, b * H:(b + 1) * H],
                                            in0=ps[:, 0:H], in1=bemb_sb[:],
                                            op=OP.add)

                # edge-attr aggregates -> e16
                ea_sb = pr1.tile([128, G, ED], f32)
                nc.sync.dma_start(out=ea_sb[:],
                                  in_=eattr_d[:].rearrange("p (g e) -> p g e", e=ED))
                sqt = pr1.tile([128, G, ED], f32)
                for b in range(NB):
                    Db, ob = D[b], offs[b]
                    eb = ea_sb[:, ob:ob + Db, :].rearrange("p g e -> p e g")
                    esum = prp.tile([128, ED], f32, tag="esum")
                    emean = prp.tile([128, ED], f32, tag="emean")
                    ess = prp.tile([128, ED], f32, tag="ess")
                    tmp = prp.tile([128, ED], f32, tag="etmp")
                    tmp2 = prp.tile([128, ED], f32, tag="etmp2")
                    nc.vector.tensor_reduce(out=e16[:, b * 16 + 4:b * 16 + 8],
                                            in_=eb, axis=AX.X, op=OP.min)
                    nc.vector.tensor_reduce(out=e16[:, b * 16 + 8:b * 16 + 12],
                                            in_=eb, axis=AX.X, op=OP.max)
                    nc.vector.tensor_reduce(out=esum[:], in_=eb, axis=AX.X,
                                            op=OP.add)
                    e0 = ea_sb[:, ob:ob + 1, :].rearrange("p g e -> p (g e)")
                    nc.vector.tensor_scalar(out=tmp[:], in0=e0,
                                            scalar1=cst(K_NPAD, b), scalar2=None,
                                            op0=OP.mult)
                    nc.vector.tensor_tensor(out=esum[:], in0=esum[:], in1=tmp[:],
                                            op=OP.subtract)
                    nc.vector.tensor_scalar(out=emean[:], in0=esum[:],
                                            scalar1=cst(K_INVDEG, b),
                                            scalar2=None, op0=OP.mult)
                    nc.vector.tensor_copy(out=e16[:, b * 16:b * 16 + 4],
                                          in_=emean[:])
                    nc.scalar.activation(sqt[:, ob:ob + Db, :],
                                         ea_sb[:, ob:ob + Db, :], AF.Square)
                    nc.vector.tensor_reduce(
                        out=ess[:],
                        in_=sqt[:, ob:ob + Db, :].rearrange("p g e -> p e g"),
                        axis=AX.X, op=OP.add)
                    e0sq = sqt[:, ob:ob + 1, :].rearrange("p g e -> p (g e)")
                    nc.vector.tensor_scalar(out=tmp[:], in0=e0sq,
                                            scalar1=cst(K_NPAD, b), scalar2=None,
                                            op0=OP.mult)
                    nc.vector.tensor_tensor(out=ess[:], in0=ess[:], in1=tmp[:],
                                            op=OP.subtract)
                    nc.scalar.activation(tmp2[:], emean[:], AF.Square)
                    nc.vector.scalar_tensor_tensor(out=tmp[:], in0=ess[:],
                                                   scalar=cst(K_INVDEG, b),
                                                   in1=tmp2[:], op0=OP.mult,
                                                   op1=OP.subtract)
                    nc.vector.tensor_scalar_max(out=tmp[:], in0=tmp[:],
                                                scalar1=0.0)
                    nc.scalar.activation(e16[:, b * 16 + 12:b * 16 + 16], tmp[:],
                                         AF.Sqrt, bias=c_eps[:])

            # ================= message-passing layers ======================
            with tc.tile_pool(name="gath", bufs=2) as gp, \
                 tc.tile_pool(name="apool", bufs=2) as apl, \
                 tc.tile_pool(name="tsqp", bufs=1) as tqp, \
                 tc.tile_pool(name="psagg", bufs=2, space="PSUM") as psa:
                CG = CG_ENV  # slot-groups per dma_gather chunk (ring budget)
                gsem = nc.alloc_semaphore("gsem") if PREP else None
                for l in range(NL):
                    stats_ps = pss.tile([1, 160], f32, tag="stats")
                    # bf16 [h ; h^2] table for the A-matmul rhs
                    tsq = tqp.tile([128, 64, 160], bf16, tag="tsq")
                    for t in range(8):
                        tmpg = sp.tile([128, 8, H], bf16, tag="tmpg")
                        nc.sync.dma_start(
                            out=tmpg[:],
                            in_=table[t * 1024:(t + 1) * 1024, 0:H]
                            .rearrange("(kc p) f -> p kc f", p=128))
                        nc.vector.tensor_copy(
                            out=tsq[:, t * 8:(t + 1) * 8, 0:H], in_=tmpg[:])
                        nc.scalar.activation(
                            tsq[:, t * 8:(t + 1) * 8, H:160], tmpg[:],
                            AF.Square)
                    for b in range(NB):
                        Db, ob = D[b], offs[b]
                        Gm = sp.tile([128, 416], f32, tag="Gm")
                        tmp = sp.tile([128, H], f32, tag="tmp")
                        tmp2 = sp.tile([128, H], f32, tag="tmp2")
                        # adjacency counts for this bucket's 128 dst lanes
                        a_sb = apl.tile([128, 64, 128], bf16, tag="a")
                        nc.sync.dma_start(
                            out=a_sb[:],
                            in_=acnt_d[:, b * 128:(b + 1) * 128]
                            .rearrange("(kc p) d -> p kc d", p=128))
                        ps_agg = psa.tile([128, 160], f32, tag="agg")
                        for kc in range(64):
                            nc.tensor.matmul(ps_agg[:], a_sb[:, kc, :],
                                             tsq[:, kc, :],
                                             start=(kc == 0), stop=(kc == 63))
                        gt = gp.tile([128, DMAX, 128], bf16, tag="gt")
                        for s0 in range(0, Db, CG):
                            cg = min(CG, Db - s0)
                            if NO_GATHER:
                                nc.vector.memset(gt[:, s0:s0 + cg, :], 0.5)
                            elif PREP:
                                nc.gpsimd.dma_gather(
                                    gt[:, s0:s0 + cg, :], table[:, :],
                                    idx_sb[:, (ob + s0) * 8:(ob + s0 + cg) * 8],
                                    cg * 128, cg * 128, 128,
                                    prepare_only=True, sem=gsem)
                                nc.gpsimd.trigger_dma(count=None)
                            else:
                                nc.gpsimd.dma_gather(
                                    gt[:, s0:s0 + cg, :], table[:, :],
                                    idx_sb[:, (ob + s0) * 8:(ob + s0 + cg) * 8],
                                    cg * 128, cg * 128, 128)
                        gv = gt[:, 0:Db, 0:H].rearrange("p g f -> p f g")
                        nc.vector.tensor_reduce(out=Gm[:, 160:240], in_=gv,
                                                axis=AX.X, op=OP.min)
                        nc.vector.tensor_reduce(out=Gm[:, 240:320], in_=gv,
                                                axis=AX.X, op=OP.max)
                        # mean / std from the A-matmul sums (exact counts)
                        nc.vector.tensor_scalar(out=Gm[:, 80:160],
                                                in0=ps_agg[:, 0:H],
                                                scalar1=cst(K_INVDEG, b),
                                                scalar2=None, op0=OP.mult)
                        nc.scalar.activation(tmp2[:], Gm[:, 80:160], AF.Square)
                        nc.vector.scalar_tensor_tensor(out=tmp[:],
                                                       in0=ps_agg[:, H:160],
                                                       scalar=cst(K_INVDEG, b),
                                                       in1=tmp2[:], op0=OP.mult,
                                                       op1=OP.subtract)
                        nc.vector.tensor_scalar_max(out=tmp[:], in0=tmp[:],
                                                    scalar1=0.0)
                        nc.scalar.activation(Gm[:, 320:400], tmp[:], AF.Sqrt,
                                             bias=c_eps[:])
                        nc.vector.tensor_scalar(out=Gm[:, 0:80],
                                                in0=h_own[:, b * H:(b + 1) * H],
                                                scalar1=cst(K_HAS, b),
                                                scalar2=None, op0=OP.mult)
                        nc.vector.tensor_copy(out=Gm[:, 400:416],
                                              in_=e16[:, b * 16:(b + 1) * 16])
                        ops = pso.tile([128, 512], f32, tag="mm")
                        for kt in range(4):
                            kw = min(128, 416 - kt * 128)
                            pt = pst.tile([128, 128], f32, tag="tr")
                            nc.tensor.transpose(pt[0:kw, :],
                                                Gm[:, kt * 128:kt * 128 + kw],
                                                ident[:])
                            gT = sp.tile([128, 128], f32, tag="gT")
                            nc.vector.tensor_copy(out=gT[0:kw, :],
                                                  in_=pt[0:kw, :])
                            nc.tensor.matmul(ops[:, 0:240], gT[0:kw, :],
                                             weff(l, kt)[0:kw, :],
                                             start=(kt == 0), stop=(kt == 3))
                        ot = o_all[:, b * H:(b + 1) * H]
                        nc.vector.tensor_tensor(out=ot, in0=ops[:, 0:80],
                                                in1=beff(l, 0), op=OP.add)
                        u = sp.tile([128, H], f32, tag="uep")
                        nc.vector.tensor_tensor(out=u[:], in0=ops[:, 80:160],
                                                in1=beff(l, 1), op=OP.add)
                        nc.vector.scalar_tensor_tensor(out=ot, in0=u[:],
                                                       scalar=cst(K_AMP, b),
                                                       in1=ot, op0=OP.mult,
                                                       op1=OP.add)
                        nc.vector.tensor_tensor(out=u[:], in0=ops[:, 160:240],
                                                in1=beff(l, 2), op=OP.add)
                        nc.vector.scalar_tensor_tensor(out=ot, in0=u[:],
                                                       scalar=cst(K_ATT, b),
                                                       in1=ot, op0=OP.mult,
                                                       op1=OP.add)
                        st = sp.tile([128, 160], f32, tag="stin")
                        nc.vector.tensor_copy(out=st[:, 0:80], in_=ot)
                        nc.scalar.activation(st[:, 80:160], ot, AF.Square)
                        nc.tensor.matmul(stats_ps[:], ones_sb[:], st[:],
                                         start=(b == 0), stop=(b == NB - 1))

                    # BN: AllReduce stats, compute scale/shift (replicated)
                    st_sb = sp.tile([1, 160], f32, tag="stsb")
                    nc.vector.tensor_copy(out=st_sb[:], in_=stats_ps[:])
                    nc.sync.dma_start(out=bn_in[:], in_=st_sb[:])
                    bn_out = bn_outs[l]
                    if NO_CC:
                        nc.sync.dma_start(out=bn_out[:], in_=bn_in[:])
                    else:
                        nc.gpsimd.collective_compute(
                            "AllReduce", OP.add, replica_groups=groups,
                            ins=[bn_in[:].opt()], outs=[bn_out[:].opt()])
                    mv1 = sp.tile([1, 160], f32, tag="mv1")
                    nc.sync.dma_start(out=mv1[:], in_=bn_out[:])
                    mv = sp.tile([128, 160], f32, tag="mv")
                    if NO_PB:
                        nc.sync.dma_start(
                            out=mv[:],
                            in_=bn_out[:].to_broadcast((128, 160)))
                    else:
                        nc.gpsimd.partition_broadcast(mv[:], mv1[:])
                    mu = sp.tile([128, H], f32, tag="mu")
                    nc.vector.tensor_scalar(out=mu[:], in0=mv[:, 0:80],
                                            scalar1=1.0 / N, scalar2=None,
                                            op0=OP.mult)
                    ex2 = sp.tile([128, H], f32, tag="ex2")
                    nc.vector.tensor_scalar(out=ex2[:], in0=mv[:, 80:160],
                                            scalar1=1.0 / N, scalar2=None,
                                            op0=OP.mult)
                    musq = sp.tile([128, H], f32, tag="musq")
                    nc.scalar.activation(musq[:], mu[:], AF.Square)
                    var = sp.tile([128, H], f32, tag="var")
                    nc.vector.tensor_tensor(out=var[:], in0=ex2[:], in1=musq[:],
                                            op=OP.subtract)
                    nc.scalar.activation(var[:], var[:], AF.Sqrt,
                                         bias=c_eps[:])
                    rinv = sp.tile([128, H], f32, tag="rinv")
                    nc.vector.reciprocal(rinv[:], var[:])
                    grinv = sp.tile([128, H], f32, tag="grinv")
                    nc.vector.tensor_tensor(out=grinv[:], in0=rinv[:],
                                            in1=bn_sb[:, l * 80:(l + 1) * 80],
                                            op=OP.mult)
                    beta = bn_sb[:, (NL + l) * 80:(NL + l + 1) * 80]
                    for b in range(NB):
                        ot = o_all[:, b * H:(b + 1) * H]
                        hb = h_own[:, b * H:(b + 1) * H]
                        t1 = sp.tile([128, H], f32, tag="t1")
                        nc.vector.tensor_tensor(out=t1[:], in0=ot, in1=mu[:],
                                                op=OP.subtract)
                        nc.vector.tensor_tensor(out=t1[:], in0=t1[:],
                                                in1=grinv[:], op=OP.mult)
                        nc.vector.tensor_tensor(out=t1[:], in0=t1[:], in1=beta,
                                                op=OP.add)
                        nc.vector.tensor_scalar_max(out=t1[:], in0=t1[:],
                                                    scalar1=0.0)
                        nc.vector.tensor_tensor(out=hb, in0=t1[:], in1=hb,
                                                op=OP.add)
                        if l < NL - 1:
                            hb16 = sp.tile([128, H], bf16, tag="hb16")
                            nc.vector.tensor_copy(out=hb16[:], in_=hb)
                            nc.sync.dma_start(
                                out=hnew_bounce[b * 128:(b + 1) * 128, 0:H],
                                in_=hb16[:])
                        else:
                            mlp_bucket(b)
                    if l < NL - 1:
                        if NO_CC:
                            for cc in range(NC_):
                                nc.sync.dma_start(
                                    out=table[cc * NPC:(cc + 1) * NPC, :],
                                    in_=hnew_bounce[:])
                        else:
                            nc.gpsimd.collective_compute(
                                "AllGather", OP.bypass, replica_groups=groups,
                                ins=[hnew_bounce[:].opt()],
                                outs=[table[0:N, :].opt()])

            # ================= distance phase ==============================
            with tc.tile_pool(name="fin", bufs=1) as fp, \
                 tc.tile_pool(name="span", bufs=1) as spp, \
                 tc.tile_pool(name="ospan", bufs=2) as osp:
                nc.sync.dma_start(out=ct_bounce[:], in_=ct_own[:])
                if NO_CC:
                    for cc in range(NC_):
                        nc.sync.dma_start(
                            out=ct_gath[cc * 34:(cc + 1) * 34, :],
                            in_=ct_bounce[:])
                else:
                    nc.gpsimd.collective_compute(
                        "AllGather", OP.bypass, replica_groups=groups,
                        ins=[ct_bounce[:].opt()], outs=[ct_gath[:].opt()])
                rhs_all = fp.tile([34, N], bf16)
                for c in range(NC_):
                    nc.sync.dma_start(out=rhs_all[:, c * NPC:(c + 1) * NPC],
                                      in_=ct_gath[c * 34:(c + 1) * 34, :])

                MB2 = 4  # row-chunks per activation-table-switch phase
                for m0 in range(0, NPC // 128, MB2):
                    spans = []
                    for mi in range(MB2):
                        m = m0 + mi
                        span = spp.tile([128, N], bf16, tag=f"span{mi}")
                        spans.append(span)
                        for t in range(N // 512):
                            psd = pso.tile([128, 512], f32, tag="mm")
                            nc.tensor.matmul(psd[:],
                                             rt_own[:, m * 128:(m + 1) * 128],
                                             rhs_all[:, t * 512:(t + 1) * 512],
                                             start=True, stop=True)
                            nc.vector.tensor_scalar_max(
                                out=span[:, t * 512:(t + 1) * 512], in0=psd[:],
                                scalar1=0.0)
                    for mi in range(MB2):
                        nc.scalar.activation(spans[mi][:], spans[mi][:], AF.Ln,
                                             bias=c_tiny[:])
                    for mi in range(MB2):
                        m = m0 + mi
                        for hlf in range(4):
                            sl = slice(hlf * (N // 4), (hlf + 1) * (N // 4))
                            ot = osp.tile([128, N // 4], f32, tag="ospan")
                            nc.scalar.activation(ot[:], spans[mi][:, sl],
                                                 AF.Sigmoid, scale=-B_UMAP,
                                                 bias=c_nlna[:])
                            nc.sync.dma_start(
                                out=out_d[m * 128:(m + 1) * 128, sl],
                                in_=ot[:])

    nc.compile()
    return nc


# --------------------------------------------------------------------------
# entry point
# --------------------------------------------------------------------------
def get_program(D):
    import os
    key = (tuple(D), os.environ.get("NO_GATHER"), os.environ.get("NO_CC"),
           os.environ.get("NO_PB"), os.environ.get("GATHER_CG"),
           os.environ.get("GATHER_PREP"))
    if key not in _cache:
        _cache[key] = _build(D)
    return _cache[key]


def make_in_maps(inputs):
    shared, per_core, perm_nodes, D = _prepare(inputs)
    in_maps = []
    for c in range(NC_):
        m = dict(shared)
        m.update(per_core[c])
        in_maps.append(m)
    return in_maps, perm_nodes, D


def kernel(**inputs):
    from concourse.bass_utils import run_bass_kernel_spmd

    in_maps, perm_nodes, D = make_in_maps(inputs)
    nc = get_program(D)
    res = run_bass_kernel_spmd(nc, in_maps, list(range(NC_)))
    dev = np.concatenate([res.results[c]["out"] for c in range(NC_)], axis=0)

    out = np.empty((N, N), dtype=np.float32)
    out[np.ix_(perm_nodes, perm_nodes)] = dev
    return out



# revision 33
# speedup vs baseline: 20.7376x; 1.0071x over previous
"""Trainium2 Bass kernel for nn_Net_41807211660013 (PNA-style GNN + UMAP head).

Contract: kernel(**inputs) takes FULL unsharded inputs (as from
reference.setup_inputs()) and returns the FULL [8192, 8192] float32 output.

Strategy (8 NeuronCores, SPMD):
  - nodes sharded by id range: core c owns dst nodes [1024c, 1024(c+1))
  - host reorders each core's nodes by degree into 8 buckets of 128 lanes with
    per-bucket uniform padded degree D_b (pad slots repeat a real neighbor);
    all gather indices are in this permuted order
  - per layer: dma_gather of bf16 h rows (256B) from an HBM table
    [N+1, 128] -> [128, D_b, 128] tiles feed segmented min/max via DVE
    tensor_reduce; sum/sumsq come from a PE matmul of host-built bf16
    adjacency counts against a bf16 [h ; h^2] copy of the table (exact
    counts, no padding corrections); folded W_post matmul on PE; BatchNorm
    stats via ones-matmul + AllReduce (Shared dram); residual; the h-shard
    AllGather writes the bf16 table directly (contiguous 256B rows)
  - final: 3-layer MLP inlined into the last layer's BN loop, AllGather of
    augmented y^T (bf16), row-sharded NxN bf16 distance matmul with
    p = sigmoid(-B*ln(relu(d2)+tiny) - ln(A))
  - host un-permutes output rows/cols
"""
import sys

if "/opt/trn_rl_repo" not in sys.path:
    sys.path.insert(0, "/opt/trn_rl_repo")

import numpy as np

N, E, F_IN, H, ED = 8192, 524288, 39, 80, 4
A_UMAP, B_UMAP = 0.583, 1.334
NC_ = 8
NPC = N // NC_            # 1024 nodes per core
NB = NPC // 128           # 8 buckets
ZERO_ROW = N              # table row of zeros
TABLE_ROWS = N + 1
NL = 4                    # message-passing layers
SQRT_EPS = float(np.sqrt(np.float32(1e-5)))
LN_A = float(np.log(np.float32(A_UMAP)))

# const layout indices (consts tile is [128, 5*NB], slice [:, k*NB+b])
K_INVDEG, K_NPAD, K_AMP, K_ATT, K_HAS = range(5)

_cache = {}


# --------------------------------------------------------------------------
# host preprocessing
# --------------------------------------------------------------------------
def _prepare(inputs):
    x = np.asarray(inputs["x"], np.float32)
    edge_attr = np.asarray(inputs["edge_attr"], np.float32)
    edge_index = np.asarray(inputs["edge_index"], np.int64)
    src_arr, dst_arr = edge_index[0], edge_index[1]

    deg = np.bincount(dst_arr, minlength=N).astype(np.float32)
    logd = np.log1p(deg)
    avg_log = logd.mean(dtype=np.float32)
    amp = (logd / avg_log).astype(np.float32)
    att = np.where(logd > 0, avg_log / np.where(logd > 0, logd, 1.0), 1.0).astype(np.float32)
    has = (deg > 0).astype(np.float32)
    inv_deg = (1.0 / np.where(deg > 0, deg, 1.0)).astype(np.float32)

    order = np.argsort(dst_arr, kind="stable")
    sorted_src = src_arr[order]
    sorted_eid = order
    starts = np.searchsorted(dst_arr[order], np.arange(N))
    ends = np.searchsorted(dst_arr[order], np.arange(N) + 1)

    perm_nodes = np.empty(N, dtype=np.int64)
    for c in range(NC_):
        own = np.arange(c * NPC, (c + 1) * NPC)
        loc = own[np.argsort(-deg[own], kind="stable")]
        perm_nodes[c * NPC:(c + 1) * NPC] = loc
    perm_pos = np.empty(N, dtype=np.int64)
    perm_pos[perm_nodes] = np.arange(N)

    deg_perm = deg[perm_nodes].astype(np.int64).reshape(NC_, NB, 128)
    D = np.maximum(deg_perm.max(axis=(0, 2)), 1).astype(np.int64)  # [NB]
    G = int(D.sum())

    # slot structures per core
    idx_slots = np.full((NC_, G, 128), ZERO_ROW, dtype=np.int32)   # [c][slot][lane]
    eattr_p = np.zeros((NC_, 128, G, ED), dtype=np.float32)        # partition-major
    npad = np.zeros(N, dtype=np.float32)
    offs = np.concatenate([[0], np.cumsum(D)]).astype(np.int64)
    for c in range(NC_):
        for b in range(NB):
            Db, ob = int(D[b]), int(offs[b])
            for p in range(128):
                r = c * NPC + b * 128 + p
                n = perm_nodes[r]
                d = int(deg[n])
                if d == 0:
                    continue
                srcs = perm_pos[sorted_src[starts[n]:ends[n]]]
                eids = sorted_eid[starts[n]:ends[n]]
                idx_slots[c, ob:ob + d, p] = srcs
                idx_slots[c, ob + d:ob + Db, p] = srcs[0]
                ea = edge_attr[eids]
                eattr_p[c, p, ob:ob + d] = ea
                eattr_p[c, p, ob + d:ob + Db] = ea[0]
                npad[r] = Db - d

    # idx in dma_gather wrap layout: value for slot i lives at [i % 16, i // 16],
    # replicated over all 128 partitions
    idx_wrap = np.zeros((NC_, 128, G * 8), dtype=np.int16)
    for c in range(NC_):
        flat = idx_slots[c].reshape(G * 128)             # i = g*128 + lane
        w = flat.reshape(G * 8, 16).T.astype(np.int16)   # [16, G*8]
        idx_wrap[c] = np.tile(w, (8, 1))

    # per-(core, lane, bucket) consts [128, 5*NB]
    consts = np.zeros((NC_, 128, 5 * NB), dtype=np.float32)
    for c in range(NC_):
        rows = perm_nodes[c * NPC:(c + 1) * NPC].reshape(NB, 128)
        trows = np.arange(c * NPC, (c + 1) * NPC).reshape(NB, 128)
        for b in range(NB):
            consts[c, :, K_INVDEG * NB + b] = inv_deg[rows[b]]
            consts[c, :, K_NPAD * NB + b] = npad[trows[b]]
            consts[c, :, K_AMP * NB + b] = amp[rows[b]]
            consts[c, :, K_ATT * NB + b] = att[rows[b]]
            consts[c, :, K_HAS * NB + b] = has[rows[b]]

    # folded W_post weights
    W_post = np.asarray(inputs["W_post"], np.float32)
    b_post = np.asarray(inputs["b_post"], np.float32)
    W_eff = np.zeros((NL, 416, 240), dtype=np.float32)
    b_eff = np.zeros((NL, 3, 80), dtype=np.float32)
    for l in range(NL):
        W = W_post[l]
        for v in range(3):
            o = 656 * v
            W_hown = W[o + 0:o + 80] + W[o + 164:o + 244] + W[o + 328:o + 408]
            W_e = np.concatenate([W[o + 160:o + 164], W[o + 324:o + 328],
                                  W[o + 488:o + 492], W[o + 652:o + 656]], axis=0)
            W_eff[l, :, 80 * v:80 * (v + 1)] = np.concatenate(
                [W_hown, W[o + 80:o + 160], W[o + 244:o + 324],
                 W[o + 408:o + 488], W[o + 572:o + 652], W_e], axis=0)
            b_eff[l, v] = np.float32(SQRT_EPS) * W[o + 492:o + 572].sum(axis=0)
        b_eff[l, 0] += b_post[l]
    # pack for device: [128, NL, 4, 240] (K-chunk partition-major, zero padded)
    w_eff_packed = np.zeros((128, NL, 4, 240), dtype=np.float32)
    for l in range(NL):
        for kt in range(4):
            kw = min(128, 416 - kt * 128)
            w_eff_packed[:kw, l, kt] = W_eff[l, kt * 128:kt * 128 + kw]
    b_eff_r = np.broadcast_to(b_eff.reshape(1, NL * 3 * 80), (128, NL * 3 * 80)).copy()

    bn_gamma = np.asarray(inputs["bn_gamma"], np.float32)
    bn_beta = np.asarray(inputs["bn_beta"], np.float32)
    bn_r = np.broadcast_to(
        np.concatenate([bn_gamma.reshape(NL * 80), bn_beta.reshape(NL * 80)])
        .reshape(1, 2 * NL * 80), (128, 2 * NL * 80)).copy()

    xt = x.T[:, perm_nodes].copy()                 # [39, 8192] table order

    mlp_bT = np.zeros((128, 3), np.float32)
    mlp_bT[0:40, 0] = np.asarray(inputs["b1"], np.float32)
    mlp_bT[0:20, 1] = np.asarray(inputs["b2"], np.float32)
    mlp_bT[0:32, 2] = np.asarray(inputs["b3"], np.float32)

    import ml_dtypes
    shared = dict(
        xt=np.ascontiguousarray(xt).astype(ml_dtypes.bfloat16),
        w_emb=np.asarray(inputs["W_emb"], np.float32).astype(ml_dtypes.bfloat16),
        b_emb_r=np.broadcast_to(np.asarray(inputs["b_emb"], np.float32)
                                .reshape(1, 80), (128, 80)).copy(),
        w_eff=w_eff_packed.reshape(128, NL * 4 * 240),
        b_eff_r=b_eff_r,
        bn_r=bn_r,
        w1=np.asarray(inputs["W1"], np.float32),
        w2=np.asarray(inputs["W2"], np.float32),
        w3=np.asarray(inputs["W3"], np.float32),
        mlp_b_r=mlp_bT,
    )
    # per-core adjacency counts A[src_perm_row, dst_lane] (for PE sum/sumsq)
    import ml_dtypes
    acnt = np.zeros((NC_, N, NPC), dtype=np.float32)
    src_pos = perm_pos[src_arr]
    dst_pos = perm_pos[dst_arr]
    cores = dst_pos // NPC
    lanes = dst_pos % NPC
    for c in range(NC_):
        m = cores == c
        np.add.at(acnt[c], (src_pos[m], lanes[m]), 1.0)

    per_core = []
    for c in range(NC_):
        per_core.append(dict(
            xt_own=np.ascontiguousarray(xt[:, c * NPC:(c + 1) * NPC])
            .astype(ml_dtypes.bfloat16),
            idx=idx_wrap[c],
            eattr=eattr_p[c].reshape(128, G * ED),
            consts=consts[c],
            acnt=acnt[c].astype(ml_dtypes.bfloat16),
        ))
    return shared, per_core, perm_nodes, [int(d) for d in D]


# --------------------------------------------------------------------------
# device program
# --------------------------------------------------------------------------
def _build(D):
    import os
    NO_GATHER = os.environ.get("NO_GATHER") == "1"
    NO_CC = os.environ.get("NO_CC") == "1"
    NO_PB = os.environ.get("NO_PB") == "1"
    CG_ENV = int(os.environ.get("GATHER_CG", "7"))
    PREP = os.environ.get("GATHER_PREP") == "1"
    import concourse.bass as bass  # noqa: F401
    import concourse.bacc as bacc
    import concourse.tile as tile
    import concourse.mybir as mybir
    import concourse.masks as masks

    f32 = mybir.dt.float32
    bf16 = mybir.dt.bfloat16
    i16 = mybir.dt.int16
    AF = mybir.ActivationFunctionType
    OP = mybir.AluOpType
    AX = mybir.AxisListType

    G = sum(D)
    offs = [0]
    for d in D:
        offs.append(offs[-1] + d)
    DMAX = max(D)

    nc = bacc.Bacc("TRN2", target_bir_lowering=False, debug=False,
                   num_devices=NC_)

    # ---- I/O ----
    xt_d = nc.dram_tensor("xt", [F_IN, N], bf16, kind="ExternalInput")
    xt_own_d = nc.dram_tensor("xt_own", [F_IN, NPC], bf16, kind="ExternalInput")
    w_emb_d = nc.dram_tensor("w_emb", [F_IN, H], bf16, kind="ExternalInput")
    b_emb_d = nc.dram_tensor("b_emb_r", [128, H], f32, kind="ExternalInput")
    idx_d = nc.dram_tensor("idx", [128, G * 8], i16, kind="ExternalInput")
    eattr_d = nc.dram_tensor("eattr", [128, G * ED], f32, kind="ExternalInput")
    consts_d = nc.dram_tensor("consts", [128, 5 * NB], f32, kind="ExternalInput")
    w_eff_d = nc.dram_tensor("w_eff", [128, NL * 4 * 240], f32, kind="ExternalInput")
    b_eff_d = nc.dram_tensor("b_eff_r", [128, NL * 3 * 80], f32, kind="ExternalInput")
    bn_d = nc.dram_tensor("bn_r", [128, 2 * NL * 80], f32, kind="ExternalInput")
    w1_d = nc.dram_tensor("w1", [80, 40], f32, kind="ExternalInput")
    w2_d = nc.dram_tensor("w2", [40, 20], f32, kind="ExternalInput")
    w3_d = nc.dram_tensor("w3", [20, 32], f32, kind="ExternalInput")
    mlpb_d = nc.dram_tensor("mlp_b_r", [128, 3], f32, kind="ExternalInput")
    acnt_d = nc.dram_tensor("acnt", [N, NPC], bf16, kind="ExternalInput")
    out_d = nc.dram_tensor("out", [NPC, N], f32, kind="ExternalOutput")

    groups = [list(range(NC_))]

    with tile.TileContext(nc) as tc:
        with (
            tc.tile_pool(name="persist", bufs=1) as pp,
            tc.tile_pool(name="small", bufs=2) as sp,
            tc.tile_pool(name="psum_t", bufs=2, space="PSUM") as pst,
            tc.tile_pool(name="psum_mm", bufs=3, space="PSUM") as pso,
            tc.tile_pool(name="psum_st", bufs=1, space="PSUM") as pss,
            tc.tile_pool(name="dram", bufs=1, space="DRAM") as dp,
        ):
            # ---- internal DRAM (pool tiles => dependency-tracked) ----
            table = dp.tile([TABLE_ROWS, 128], bf16)
            hnew_bounce = dp.tile([NPC, 128], bf16)
            bn_in = dp.tile([1, 160], f32)
            bn_outs = [dp.tile([1, 160], f32, addr_space="Shared",
                               name=f"bn_out{l}") for l in range(NL)]
            ct_bounce = dp.tile([34, NPC], bf16)
            ct_gath = dp.tile([NC_ * 34, NPC], bf16, addr_space="Shared")

            # ---- persistent tiles ----
            idx_sb = pp.tile([128, G * 8], i16)
            nc.sync.dma_start(out=idx_sb[:], in_=idx_d[:])
            consts_sb = pp.tile([128, 5 * NB], f32)
            nc.sync.dma_start(out=consts_sb[:], in_=consts_d[:])
            weff_sb = pp.tile([128, NL * 4 * 240], f32)
            nc.sync.dma_start(out=weff_sb[:], in_=w_eff_d[:])
            beff_sb = pp.tile([128, NL * 3 * 80], f32)
            nc.sync.dma_start(out=beff_sb[:], in_=b_eff_d[:])
            bn_sb = pp.tile([128, 2 * NL * 80], f32)
            nc.sync.dma_start(out=bn_sb[:], in_=bn_d[:])
            ident = pp.tile([128, 128], f32)
            masks.make_identity(nc, ident[:])
            ones_sb = pp.tile([128, 1], f32)
            nc.vector.memset(ones_sb[:], 1.0)
            h_own = pp.tile([128, NB * H], f32)
            o_all = pp.tile([128, NB * H], f32)
            e16 = pp.tile([128, NB * 16], f32)
            zrow = pp.tile([1, 128], bf16)
            nc.vector.memset(zrow[:], 0.0)
            c_eps = pp.tile([128, 1], f32)
            nc.vector.memset(c_eps[:], 1e-5)
            c_tiny = pp.tile([128, 1], f32)
            nc.vector.memset(c_tiny[:], 1e-30)
            c_nlna = pp.tile([128, 1], f32)
            nc.vector.memset(c_nlna[:], -LN_A)
            w1_sb = pp.tile([80, 40], f32)
            nc.sync.dma_start(out=w1_sb[:], in_=w1_d[:])
            w2_sb = pp.tile([40, 20], f32)
            nc.sync.dma_start(out=w2_sb[:], in_=w2_d[:])
            w3_sb = pp.tile([20, 32], f32)
            nc.sync.dma_start(out=w3_sb[:], in_=w3_d[:])
            mlpb_sb = pp.tile([128, 3], f32)
            nc.sync.dma_start(out=mlpb_sb[:], in_=mlpb_d[:])
            ct_own = pp.tile([34, NPC], bf16)
            rt_own = pp.tile([34, NPC], bf16)
            ones16 = pp.tile([1, NPC], bf16)
            nc.vector.memset(ones16[:], 1.0)
            nc.sync.dma_start(out=ct_own[33:34, :], in_=ones16[:])
            nc.sync.dma_start(out=rt_own[32:33, :], in_=ones16[:])

            def mlp_bucket(b):
                hb = h_own[:, b * H:(b + 1) * H]
                ptr = pst.tile([128, 128], f32, tag="tr")
                nc.tensor.transpose(ptr[0:H, :], hb, ident[:])
                hT = sp.tile([80, 128], f32, tag="hT")
                nc.vector.tensor_copy(out=hT[:], in_=ptr[0:H, :])
                ps1 = pso.tile([128, 512], f32, tag="mm")
                nc.tensor.matmul(ps1[0:40, 0:128], w1_sb[:], hT[:],
                                 start=True, stop=True)
                y1T = sp.tile([40, 128], f32, tag="y1T")
                nc.vector.tensor_scalar(out=y1T[:], in0=ps1[0:40, 0:128],
                                        scalar1=mlpb_sb[0:40, 0:1],
                                        scalar2=0.0, op0=OP.add, op1=OP.max)
                ps2 = pso.tile([128, 512], f32, tag="mm")
                nc.tensor.matmul(ps2[0:20, 0:128], w2_sb[:], y1T[:],
                                 start=True, stop=True)
                y2T = sp.tile([20, 128], f32, tag="y2T")
                nc.vector.tensor_scalar(out=y2T[:], in0=ps2[0:20, 0:128],
                                        scalar1=mlpb_sb[0:20, 1:2],
                                        scalar2=0.0, op0=OP.add, op1=OP.max)
                ps3 = pso.tile([128, 512], f32, tag="mm")
                nc.tensor.matmul(ps3[0:32, 0:128], w3_sb[:], y2T[:],
                                 start=True, stop=True)
                y3T = sp.tile([32, 128], f32, tag="y3T")
                nc.vector.tensor_scalar(out=y3T[:], in0=ps3[0:32, 0:128],
                                        scalar1=mlpb_sb[0:32, 2:3],
                                        scalar2=None, op0=OP.add)
                y3sqT = sp.tile([32, 128], f32, tag="y3sqT")
                nc.scalar.activation(y3sqT[:], y3T[:], AF.Square)
                ps4 = pso.tile([128, 512], f32, tag="mm")
                nc.tensor.matmul(ps4[0:1, 0:128], ones_sb[0:32, :],
                                 y3sqT[:], start=True, stop=True)
                co = ct_own[:, b * 128:(b + 1) * 128]
                ro = rt_own[:, b * 128:(b + 1) * 128]
                sq_sb = sp.tile([1, 128], bf16, tag="sq_sb")
                nc.vector.tensor_copy(out=sq_sb[:], in_=ps4[0:1, 0:128])
                nc.vector.tensor_copy(out=co[0:32, :], in_=y3T[:])
                nc.vector.tensor_copy(out=co[32:33, :], in_=sq_sb[:])
                nc.vector.tensor_scalar(out=ro[0:32, :], in0=y3T[:],
                                        scalar1=-2.0, scalar2=None,
                                        op0=OP.mult)
                nc.sync.dma_start(out=ro[33:34, :], in_=sq_sb[:])

            def cst(k, b):
                return consts_sb[:, k * NB + b:k * NB + b + 1]

            def weff(l, kt):
                o = (l * 4 + kt) * 240
                return weff_sb[:, o:o + 240]

            def beff(l, v):
                o = (l * 3 + v) * 80
                return beff_sb[:, o:o + 80]

            # table zero row
            nc.sync.dma_start(out=table[ZERO_ROW:ZERO_ROW + 1, :], in_=zrow[:])

            # ================= prologue: embed + edge-attr aggregates ======
            with tc.tile_pool(name="prol", bufs=2) as prp, \
                 tc.tile_pool(name="prol1", bufs=1) as pr1:
                wemb_sb = pr1.tile([F_IN, H], bf16)
                nc.sync.dma_start(out=wemb_sb[:], in_=w_emb_d[:])
                bemb_sb = pr1.tile([128, H], f32)
                nc.sync.dma_start(out=bemb_sb[:], in_=b_emb_d[:])
                xt_sb = pr1.tile([F_IN, N], bf16)
                nc.sync.dma_start(out=xt_sb[:], in_=xt_d[:])
                xto_sb = pr1.tile([F_IN, NPC], bf16)
                nc.sync.dma_start(out=xto_sb[:], in_=xt_own_d[:])

                for k in range(N // 128):
                    ps = pso.tile([128, 512], f32, tag="mm")
                    nc.tensor.matmul(ps[:, 0:H], xt_sb[:, k * 128:(k + 1) * 128],
                                     wemb_sb[:], start=True, stop=True)
                    hch = prp.tile([128, H], bf16, tag="hch")
                    nc.vector.tensor_tensor(out=hch[:], in0=ps[:, 0:H],
                                            in1=bemb_sb[:], op=OP.add)
                    nc.sync.dma_start(out=table[k * 128:(k + 1) * 128, 0:H],
                                      in_=hch[:])
                for b in range(NB):
                    ps = pso.tile([128, 512], f32, tag="mm")
                    nc.tensor.matmul(ps[:, 0:H], xto_sb[:, b * 128:(b + 1) * 128],
                                     wemb_sb[:], start=True, stop=True)
                    nc.vector.tensor_tensor(out=h_own[:, b * H:(b + 1) * H],
                                            in0=ps[:, 0:H], in1=bemb_sb[:],
                                            op=OP.add)

                # edge-attr aggregates -> e16
                ea_sb = pr1.tile([128, G, ED], f32)
                nc.sync.dma_start(out=ea_sb[:],
                                  in_=eattr_d[:].rearrange("p (g e) -> p g e", e=ED))
                sqt = pr1.tile([128, G, ED], f32)
                for b in range(NB):
                    Db, ob = D[b], offs[b]
                    eb = ea_sb[:, ob:ob + Db, :].rearrange("p g e -> p e g")
                    esum = prp.tile([128, ED], f32, tag="esum")
                    emean = prp.tile([128, ED], f32, tag="emean")
                    ess = prp.tile([128, ED], f32, tag="ess")
                    tmp = prp.tile([128, ED], f32, tag="etmp")
                    tmp2 = prp.tile([128, ED], f32, tag="etmp2")
                    nc.vector.tensor_reduce(out=e16[:, b * 16 + 4:b * 16 + 8],
                                            in_=eb, axis=AX.X, op=OP.min)
                    nc.vector.tensor_reduce(out=e16[:, b * 16 + 8:b * 16 + 12],
                                            in_=eb, axis=AX.X, op=OP.max)
                    nc.vector.tensor_reduce(out=esum[:], in_=eb, axis=AX.X,
                                            op=OP.add)
                    e0 = ea_sb[:, ob:ob + 1, :].rearrange("p g e -> p (g e)")
                    nc.vector.tensor_scalar(out=tmp[:], in0=e0,
                                            scalar1=cst(K_NPAD, b), scalar2=None,
                                            op0=OP.mult)
                    nc.vector.tensor_tensor(out=esum[:], in0=esum[:], in1=tmp[:],
                                            op=OP.subtract)
                    nc.vector.tensor_scalar(out=emean[:], in0=esum[:],
                                            scalar1=cst(K_INVDEG, b),
                                            scalar2=None, op0=OP.mult)
                    nc.vector.tensor_copy(out=e16[:, b * 16:b * 16 + 4],
                                          in_=emean[:])
                    nc.scalar.activation(sqt[:, ob:ob + Db, :],
                                         ea_sb[:, ob:ob + Db, :], AF.Square)
                    nc.vector.tensor_reduce(
                        out=ess[:],
                        in_=sqt[:, ob:ob + Db, :].rearrange("p g e -> p e g"),
                        axis=AX.X, op=OP.add)
                    e0sq = sqt[:, ob:ob + 1, :].rearrange("p g e -> p (g e)")
                    nc.vector.tensor_scalar(out=tmp[:], in0=e0sq,
                                            scalar1=cst(K_NPAD, b), scalar2=None,
                                            op0=OP.mult)
                    nc.vector.tensor_tensor(out=ess[:], in0=ess[:], in1=tmp[:],
                                            op=OP.subtract)
                    nc.scalar.activation(tmp2[:], emean[:], AF.Square)
                    nc.vector.scalar_tensor_tensor(out=tmp[:], in0=ess[:],
                                                   scalar=cst(K_INVDEG, b),
                                                   in1=tmp2[:], op0=OP.mult,
                                                   op1=OP.subtract)
                    nc.vector.tensor_scalar_max(out=tmp[:], in0=tmp[:],
                                                scalar1=0.0)
                    nc.scalar.activation(e16[:, b * 16 + 12:b * 16 + 16], tmp[:],
                                         AF.Sqrt, bias=c_eps[:])

            # ================= message-passing layers ======================
            with tc.tile_pool(name="gath", bufs=2) as gp, \
                 tc.tile_pool(name="apool", bufs=2) as apl, \
                 tc.tile_pool(name="tsqp", bufs=1) as tqp, \
                 tc.tile_pool(name="psagg", bufs=2, space="PSUM") as psa:
                CG = CG_ENV  # slot-groups per dma_gather chunk (ring budget)
                gsem = nc.alloc_semaphore("gsem") if PREP else None
                for l in range(NL):
                    stats_ps = pss.tile([1, 160], f32, tag="stats")
                    # bf16 [h ; h^2] table for the A-matmul rhs
                    tsq = tqp.tile([128, 64, 160], bf16, tag="tsq")
                    for t in range(8):
                        tmpg = sp.tile([128, 8, H], bf16, tag="tmpg")
                        nc.sync.dma_start(
                            out=tmpg[:],
                            in_=table[t * 1024:(t + 1) * 1024, 0:H]
                            .rearrange("(kc p) f -> p kc f", p=128))
                        nc.vector.tensor_copy(
                            out=tsq[:, t * 8:(t + 1) * 8, 0:H], in_=tmpg[:])
                        nc.scalar.activation(
                            tsq[:, t * 8:(t + 1) * 8, H:160], tmpg[:],
                            AF.Square)
                    for b in range(NB):
                        Db, ob = D[b], offs[b]
                        Gm = sp.tile([128, 416], f32, tag="Gm")
                        tmp = sp.tile([128, H], f32, tag="tmp")
                        tmp2 = sp.tile([128, H], f32, tag="tmp2")
                        # adjacency counts for this bucket's 128 dst lanes
                        a_sb = apl.tile([128, 64, 128], bf16, tag="a")
                        nc.sync.dma_start(
                            out=a_sb[:],
                            in_=acnt_d[:, b * 128:(b + 1) * 128]
                            .rearrange("(kc p) d -> p kc d", p=128))
                        ps_agg = psa.tile([128, 160], f32, tag="agg")
                        for kc in range(64):
                            nc.tensor.matmul(ps_agg[:], a_sb[:, kc, :],
                                             tsq[:, kc, :],
                                             start=(kc == 0), stop=(kc == 63))
                        gt = gp.tile([128, DMAX, 128], bf16, tag="gt")
                        for s0 in range(0, Db, CG):
                            cg = min(CG, Db - s0)
                            if NO_GATHER:
                                nc.vector.memset(gt[:, s0:s0 + cg, :], 0.5)
                            elif PREP:
                                nc.gpsimd.dma_gather(
                                    gt[:, s0:s0 + cg, :], table[:, :],
                                    idx_sb[:, (ob + s0) * 8:(ob + s0 + cg) * 8],
                                    cg * 128, cg * 128, 128,
                                    prepare_only=True, sem=gsem)
                                nc.gpsimd.trigger_dma(count=None)
                            else:
                                nc.gpsimd.dma_gather(
                                    gt[:, s0:s0 + cg, :], table[:, :],
                                    idx_sb[:, (ob + s0) * 8:(ob + s0 + cg) * 8],
                                    cg * 128, cg * 128, 128)
                        gv = gt[:, 0:Db, 0:H].rearrange("p g f -> p f g")
                        nc.vector.tensor_reduce(out=Gm[:, 160:240], in_=gv,
                                                axis=AX.X, op=OP.min)
                        nc.vector.tensor_reduce(out=Gm[:, 240:320], in_=gv,
                                                axis=AX.X, op=OP.max)
                        # mean / std from the A-matmul sums (exact counts)
                        nc.vector.tensor_scalar(out=Gm[:, 80:160],
                                                in0=ps_agg[:, 0:H],
                                                scalar1=cst(K_INVDEG, b),
                                                scalar2=None, op0=OP.mult)
                        nc.scalar.activation(tmp2[:], Gm[:, 80:160], AF.Square)
                        nc.vector.scalar_tensor_tensor(out=tmp[:],
                                                       in0=ps_agg[:, H:160],
                                                       scalar=cst(K_INVDEG, b),
                                                       in1=tmp2[:], op0=OP.mult,
                                                       op1=OP.subtract)
                        nc.vector.tensor_scalar_max(out=tmp[:], in0=tmp[:],
                                                    scalar1=0.0)
                        nc.scalar.activation(Gm[:, 320:400], tmp[:], AF.Sqrt,
                                             bias=c_eps[:])
                        nc.vector.tensor_scalar(out=Gm[:, 0:80],
                                                in0=h_own[:, b * H:(b + 1) * H],
                                                scalar1=cst(K_HAS, b),
                                                scalar2=None, op0=OP.mult)
                        nc.vector.tensor_copy(out=Gm[:, 400:416],
                                              in_=e16[:, b * 16:(b + 1) * 16])
                        ops = pso.tile([128, 512], f32, tag="mm")
                        for kt in range(4):
                            kw = min(128, 416 - kt * 128)
                            pt = pst.tile([128, 128], f32, tag="tr")
                            nc.tensor.transpose(pt[0:kw, :],
                                                Gm[:, kt * 128:kt * 128 + kw],
                                                ident[:])
                            gT = sp.tile([128, 128], f32, tag="gT")
                            nc.vector.tensor_copy(out=gT[0:kw, :],
                                                  in_=pt[0:kw, :])
                            nc.tensor.matmul(ops[:, 0:240], gT[0:kw, :],
                                             weff(l, kt)[0:kw, :],
                                             start=(kt == 0), stop=(kt == 3))
                        ot = o_all[:, b * H:(b + 1) * H]
                        nc.vector.tensor_tensor(out=ot, in0=ops[:, 0:80],
                                                in1=beff(l, 0), op=OP.add)
                        u = sp.tile([128, H], f32, tag="uep")
                        nc.vector.tensor_tensor(out=u[:], in0=ops[:, 80:160],
                                                in1=beff(l, 1), op=OP.add)
                        nc.vector.scalar_tensor_tensor(out=ot, in0=u[:],
                                                       scalar=cst(K_AMP, b),
                                                       in1=ot, op0=OP.mult,
                                                       op1=OP.add)
                        nc.vector.tensor_tensor(out=u[:], in0=ops[:, 160:240],
                                                in1=beff(l, 2), op=OP.add)
                        nc.vector.scalar_tensor_tensor(out=ot, in0=u[:],
                                                       scalar=cst(K_ATT, b),
                                                       in1=ot, op0=OP.mult,
                                                       op1=OP.add)
                        st = sp.tile([128, 160], f32, tag="stin")
                        nc.vector.tensor_copy(out=st[:, 0:80], in_=ot)
                        nc.scalar.activation(st[:, 80:160], ot, AF.Square)
                        nc.tensor.matmul(stats_ps[:], ones_sb[:], st[:],
                                         start=(b == 0), stop=(b == NB - 1))

                    # BN: AllReduce stats, compute scale/shift (replicated)
                    st_sb = sp.tile([1, 160], f32, tag="stsb")
                    nc.vector.tensor_copy(out=st_sb[:], in_=stats_ps[:])
                    nc.sync.dma_start(out=bn_in[:], in_=st_sb[:])
                    bn_out = bn_outs[l]
                    if NO_CC:
                        nc.sync.dma_start(out=bn_out[:], in_=bn_in[:])
                    else:
                        nc.gpsimd.collective_compute(
                            "AllReduce", OP.add, replica_groups=groups,
                            ins=[bn_in[:].opt()], outs=[bn_out[:].opt()])
                    mv = sp.tile([128, 160], f32, tag="mv")
                    nc.sync.dma_start(
                        out=mv[:], in_=bn_out[:].to_broadcast((128, 160)))
                    sc = sp.tile([128, 160], f32, tag="sc")
                    nc.vector.tensor_scalar(out=sc[:], in0=mv[:],
                                            scalar1=1.0 / N, scalar2=None,
                                            op0=OP.mult)
                    mu = sc[:, 0:80]
                    musq = sp.tile([128, H], f32, tag="musq")
                    nc.scalar.activation(musq[:], mu, AF.Square)
                    var = sp.tile([128, H], f32, tag="var")
                    nc.vector.tensor_tensor(out=var[:], in0=sc[:, 80:160],
                                            in1=musq[:], op=OP.subtract)
                    nc.scalar.activation(var[:], var[:], AF.Sqrt,
                                         bias=c_eps[:])
                    rinv = sp.tile([128, H], f32, tag="rinv")
                    nc.vector.reciprocal(rinv[:], var[:])
                    grinv = sp.tile([128, H], f32, tag="grinv")
                    nc.vector.tensor_tensor(out=grinv[:], in0=rinv[:],
                                            in1=bn_sb[:, l * 80:(l + 1) * 80],
                                            op=OP.mult)
                    beta = bn_sb[:, (NL + l) * 80:(NL + l + 1) * 80]
                    # shift = beta - mu*grinv, so BN is one mult + one add
                    shift = sp.tile([128, H], f32, tag="shift")
                    nc.vector.tensor_tensor(out=shift[:], in0=mu,
                                            in1=grinv[:], op=OP.mult)
                    nc.vector.tensor_tensor(out=shift[:], in0=beta,
                                            in1=shift[:], op=OP.subtract)
                    # tile grinv/shift to [128, 640] by doubling copies
                    g8 = sp.tile([128, NB * H], f32, tag="g8")
                    s8 = sp.tile([128, NB * H], f32, tag="s8")
                    nc.vector.tensor_copy(out=g8[:, 0:80], in_=grinv[:])
                    nc.vector.tensor_copy(out=s8[:, 0:80], in_=shift[:])
                    for w in (80, 160, 320):
                        nc.vector.tensor_copy(out=g8[:, w:2 * w],
                                              in_=g8[:, 0:w])
                        nc.vector.tensor_copy(out=s8[:, w:2 * w],
                                              in_=s8[:, 0:w])
                    # fused BN + relu + residual over all 8 buckets
                    t8 = sp.tile([128, NB * H], f32, tag="t8")
                    nc.vector.tensor_tensor(out=t8[:], in0=o_all[:],
                                            in1=g8[:], op=OP.mult)
                    nc.vector.tensor_tensor(out=t8[:], in0=t8[:], in1=s8[:],
                                            op=OP.add)
                    nc.vector.tensor_scalar_max(out=t8[:], in0=t8[:],
                                                scalar1=0.0)
                    nc.vector.tensor_tensor(out=h_own[:], in0=t8[:],
                                            in1=h_own[:], op=OP.add)
                    if l < NL - 1:
                        hb16_8 = sp.tile([128, NB * H], bf16, tag="hb16_8")
                        nc.vector.tensor_copy(out=hb16_8[:], in_=h_own[:])
                        nc.sync.dma_start(
                            out=hnew_bounce[:, 0:H]
                            .rearrange("(b p) f -> p b f", p=128),
                            in_=hb16_8[:].rearrange("p (b f) -> p b f", f=H))
                    else:
                        for b in range(NB):
                            mlp_bucket(b)
                    if l < NL - 1:
                        if NO_CC:
                            for cc in range(NC_):
                                nc.sync.dma_start(
                                    out=table[cc * NPC:(cc + 1) * NPC, :],
                                    in_=hnew_bounce[:])
                        else:
                            nc.gpsimd.collective_compute(
                                "AllGather", OP.bypass, replica_groups=groups,
                                ins=[hnew_bounce[:].opt()],
                                outs=[table[0:N, :].opt()])

            # ================= distance phase ==============================
            with tc.tile_pool(name="fin", bufs=1) as fp, \
                 tc.tile_pool(name="span", bufs=1) as spp, \
                 tc.tile_pool(name="ospan", bufs=2) as osp:
                nc.sync.dma_start(out=ct_bounce[:], in_=ct_own[:])
                if NO_CC:
                    for cc in range(NC_):
                        nc.sync.dma_start(
                            out=ct_gath[cc * 34:(cc + 1) * 34, :],
                            in_=ct_bounce[:])
                else:
                    nc.gpsimd.collective_compute(
                        "AllGather", OP.bypass, replica_groups=groups,
                        ins=[ct_bounce[:].opt()], outs=[ct_gath[:].opt()])
                rhs_all = fp.tile([34, N], bf16)
                for c in range(NC_):
                    nc.sync.dma_start(out=rhs_all[:, c * NPC:(c + 1) * NPC],
                                      in_=ct_gath[c * 34:(c + 1) * 34, :])

                MB2 = 4  # row-chunks per activation-table-switch phase
                for m0 in range(0, NPC // 128, MB2):
                    spans = []
                    for mi in range(MB2):
                        m = m0 + mi
                        span = spp.tile([128, N], bf16, tag=f"span{mi}")
                        spans.append(span)
                        for t in range(N // 512):
                            psd = pso.tile([128, 512], f32, tag="mm")
                            nc.tensor.matmul(psd[:],
                                             rt_own[:, m * 128:(m + 1) * 128],
                                             rhs_all[:, t * 512:(t + 1) * 512],
                                             start=True, stop=True)
                            nc.vector.tensor_scalar_max(
                                out=span[:, t * 512:(t + 1) * 512], in0=psd[:],
                                scalar1=0.0)
                    for mi in range(MB2):
                        nc.scalar.activation(spans[mi][:], spans[mi][:], AF.Ln,
                                             bias=c_tiny[:])
                    for mi in range(MB2):
                        m = m0 + mi
                        for hlf in range(4):
                            sl = slice(hlf * (N // 4), (hlf + 1) * (N // 4))
                            ot = osp.tile([128, N // 4], f32, tag="ospan")
                            nc.scalar.activation(ot[:], spans[mi][:, sl],
                                                 AF.Sigmoid, scale=-B_UMAP,
                                                 bias=c_nlna[:])
                            nc.sync.dma_start(
                                out=out_d[m * 128:(m + 1) * 128, sl],
                                in_=ot[:])

    nc.compile()
    return nc


# --------------------------------------------------------------------------
# entry point
# --------------------------------------------------------------------------
def get_program(D):
    import os
    key = (tuple(D), os.environ.get("NO_GATHER"), os.environ.get("NO_CC"),
           os.environ.get("NO_PB"), os.environ.get("GATHER_CG"),
           os.environ.get("GATHER_PREP"))
    if key not in _cache:
        _cache[key] = _build(D)
    return _cache[key]


def make_in_maps(inputs):
    shared, per_core, perm_nodes, D = _prepare(inputs)
    in_maps = []
    for c in range(NC_):
        m = dict(shared)
        m.update(per_core[c])
        in_maps.append(m)
    return in_maps, perm_nodes, D


def kernel(**inputs):
    from concourse.bass_utils import run_bass_kernel_spmd

    in_maps, perm_nodes, D = make_in_maps(inputs)
    nc = get_program(D)
    res = run_bass_kernel_spmd(nc, in_maps, list(range(NC_)))
    dev = np.concatenate([res.results[c]["out"] for c in range(NC_)], axis=0)

    out = np.empty((N, N), dtype=np.float32)
    out[np.ix_(perm_nodes, perm_nodes)] = dev
    return out

